# revision 9
# baseline (speedup 1.0000x reference)
"""MiniTransformer layer on 8 TRN2 NeuronCores — fp8 DoubleRow edition.

Data-parallel over batch (B=8 -> one batch element per core, no collectives).
All six big matmul groups (z=x@G, v=x@WvT, scores, PV, FFN1, FFN2) run in
fp8-e4m3 with perf_mode=DoubleRow (K=256 per instruction, ~2x bf16 rate).
PSUM accumulation is f32; residuals/LN are f32, so fp8 error only enters
through matmul operands. Host-side scales keep operands in e4m3 range
(TRN e4m3 max is +-240): G*128, Wv*64, W1*128, W2*128; descaling is folded
into activation scales / stt scalars.

v2 scheduling (vs the 361us baseline):
- chunk-loop PE emission order scores(c) -> FFN1(c-1) -> PV(c) ->
  FFN2(c-1)/transposes(c): PV no longer gated on the exp drain stream and
  FFN2 not on the relu stream (head-of-line blocking in the engine FIFOs
  caused ~1.2us stalls at every chunk boundary).
- rstd = exp(-0.5*ln(var+eps)): keeps ScalarE on ONE act-table set
  ({exp, ln, relu, copy}); AF.Sqrt lives in a different set and forced
  2 ACT_TABLE_LOADs (1.3us each) per chunk.
- LN1 output (hn) is written as bf16 and the hT transposes run in bf16
  (1 PE cycle/row instead of 2 for f32).
- FFN2 is m-sequential with LN2 drained eagerly (no deferral queue); the
  last chunk splits relu drains between ScalarE and DVE and runs the LN2
  affine on DVE so the tail is not GpSimd-serialized.
- phase-1 DMAs are spread across the sync and gpsimd queues (descriptor
  issue is ~0.6-1us per dma_start on one queue and was serializing the
  startup); x_res streams in bf16 (half the bytes).
"""

import sys

try:
    import concourse.bass as bass
except ImportError:  # pragma: no cover - fallback when sitecustomize absent
    sys.path.insert(0, "/opt/trn_rl_repo")
    import concourse.bass as bass

import numpy as np
import ml_dtypes

import concourse.mybir as mybir
import concourse.tile as tile
from concourse import bacc
from concourse.bass import ts
from concourse.bass_utils import run_bass_kernel_spmd
from concourse.masks import make_identity

AF = mybir.ActivationFunctionType
ALU = mybir.AluOpType
F32 = mybir.dt.float32
BF16 = mybir.dt.bfloat16
FP8 = mybir.dt.float8e4
DR = mybir.MatmulPerfMode.DoubleRow
FP8_NP = ml_dtypes.float8_e4m3

P = 128
D = 1024
H = 2048
E = D
ND = D // P            # 8 d-tiles
NH = H // P            # 16 h-tiles
CW = 256               # s-chunk width
M2 = CW // P           # 2 m-subtiles per chunk
EPS = 1e-5
# fp8 is scale-invariant in relative error; scales are chosen only to keep
# every on-chip cast far below the TRN e4m3 Inf threshold (|x| >= 248 -> Inf,
# there is no saturating cast in the datapath).
SG = 64.0              # scale on G (=Wk^T Wq); z*SG ~ N(0,21), 11+ sigma margin
SV = 32.0              # scale on Wv; v*SV ~ N(0,18.5)
SW1 = 32.0             # scale on W1; ut holds SW1*u <= ~115 in fp8
SW = 128.0             # scale on W2 (entries bounded by 2.9 exactly)


def build_nc(S=2048):
    NS = S // P            # s-tiles
    NCH = S // CW          # chunks

    nc = bacc.Bacc("TRN2", target_bir_lowering=False, debug=False, num_devices=8)

    x_res = nc.dram_tensor("x_res", [S, D], BF16, kind="ExternalInput").ap()
    xT = nc.dram_tensor("xT", [D, S], FP8, kind="ExternalInput").ap()
    G = nc.dram_tensor("G", [D, D], FP8, kind="ExternalInput").ap()      # *SG
    WvT = nc.dram_tensor("WvT", [D, E], FP8, kind="ExternalInput").ap()  # *SV
    W1T = nc.dram_tensor("W1T", [D, H], FP8, kind="ExternalInput").ap()  # *SW1
    W2T = nc.dram_tensor("W2T", [H, D], FP8, kind="ExternalInput").ap()  # *SW
    b1c = nc.dram_tensor("b1c", [H], F32, kind="ExternalInput").ap()  # SW1*(b1+W1@be1)
    g1 = nc.dram_tensor("g1", [D], F32, kind="ExternalInput").ap()
    be1 = nc.dram_tensor("be1", [D], F32, kind="ExternalInput").ap()
    c1 = nc.dram_tensor("c1", [D], F32, kind="ExternalInput").ap()  # be1 + b2
    g2 = nc.dram_tensor("g2", [D], F32, kind="ExternalInput").ap()
    be2 = nc.dram_tensor("be2", [D], F32, kind="ExternalInput").ap()
    out = nc.dram_tensor("out", [S, D], F32, kind="ExternalOutput").ap()

    def bcast(ap_1d, n):
        return bass.AP(tensor=ap_1d.tensor, offset=ap_1d.offset, ap=[[0, P], [1, n]])

    def col(ap_1d, j):
        return ap_1d.rearrange("(a b) -> a b", b=1)[ts(j, P), :]

    with tile.TileContext(nc) as tc:
        with (
            tc.tile_pool(name="p256", bufs=2, space="PSUM") as p256,
            tc.tile_pool(name="p512", bufs=4, space="PSUM") as p512,
            tc.tile_pool(name="ptr", bufs=1, space="PSUM") as ptr,
            tc.tile_pool(name="persist", bufs=1) as persist,
            tc.tile_pool(name="scal", bufs=24) as scal,
            tc.tile_pool(name="stats", bufs=8) as stats,
            tc.tile_pool(name="xTc", bufs=2) as xTcp,
            tc.tile_pool(name="xmp", bufs=3) as xmp,
        ):
            # ---- constants ----
            ident16 = persist.tile([P, P], BF16, tag="ident", name="ident16")
            make_identity(nc, ident16)
            # ones*SV as the PV-denominator rhs: den_psum = SV * sum_k PT.
            # [P, 2, 16] so the DR pair-dim AP step is 16B (ISA: step%16==0);
            # only column 0 is used.
            ones8 = persist.tile([P, 2, 16], FP8, tag="ones", name="ones8")
            nc.vector.memset(ones8, SV)
            eps_t = persist.tile([P, 1], F32, tag="eps", name="eps_t")
            nc.vector.memset(eps_t, EPS)
            g1bc = persist.tile([P, D], BF16, tag="g1bc", name="g1bc")
            nc.gpsimd.dma_start(out=g1bc, in_=bcast(g1, D))
            c1bc = persist.tile([P, D], F32, tag="c1bc", name="c1bc")
            nc.gpsimd.dma_start(out=c1bc, in_=bcast(c1, D))
            g2bc = persist.tile([P, D], BF16, tag="g2bc", name="g2bc")
            nc.gpsimd.dma_start(out=g2bc, in_=bcast(g2, D))
            be2bc = persist.tile([P, D], BF16, tag="be2bc", name="be2bc")
            nc.gpsimd.dma_start(out=be2bc, in_=bcast(be2, D))
            b1col = []
            for n in range(NH):
                t = persist.tile([P, 1], F32, tag=f"b1c{n}", name=f"b1col{n}")
                nc.gpsimd.dma_start(out=t, in_=col(b1c, n))
                b1col.append(t)

            # ---- persistent packed tensors (dim1 = k-tile index for DR pairs) --
            zT = persist.tile([P, ND, S], FP8, tag="zT", name="zT")
            v = persist.tile([P, NS, E], FP8, tag="v", name="v")
            w1p = persist.tile([P, ND, H], FP8, tag="w1p", name="w1p")
            w2p = persist.tile([P, NH, D], FP8, tag="w2p", name="w2p")

            # ===== phase 1: zT and v, streaming xT chunk-by-chunk =====
            with (
                tc.tile_pool(name="ph1", bufs=1) as ph1,
                tc.tile_pool(name="xp1", bufs=3) as xp1,
            ):
                # weights go down the gpsimd queue, the xT stream down the
                # sync queue: descriptor issue is ~0.6-1us per dma_start on
                # an engine queue, so splitting the issue across two queues
                # roughly halves the serialization at the head of phase 1
                xTb = xT.rearrange("(j p) s -> p j s", p=P)
                Gb = G.rearrange("(j p) d -> p j d", p=P)
                Wvb = WvT.rearrange("(j p) e -> p j e", p=P)
                # first k-pair split out of the big transfers so the first
                # z/v matmuls are gated on ~300KB, not a full 1MB tensor
                gp = ph1.tile([P, ND, D], FP8, tag="gp", name="gp")
                nc.gpsimd.dma_start(out=gp[:, 0:2, :], in_=Gb[:, 0:2, :])
                xs0 = xp1.tile([P, ND, CW], FP8, tag="xs", name="xs0")
                nc.sync.dma_start(out=xs0, in_=xTb[:, :, ts(0, CW)])
                xs1 = xp1.tile([P, ND, CW], FP8, tag="xs", name="xs1")
                nc.sync.dma_start(out=xs1, in_=xTb[:, :, ts(1, CW)])
                nc.gpsimd.dma_start(out=gp[:, 2:ND, :], in_=Gb[:, 2:ND, :])
                wv_sb = ph1.tile([P, ND, E], FP8, tag="wv", name="wv")
                nc.gpsimd.dma_start(out=wv_sb[:, 0:2, :], in_=Wvb[:, 0:2, :])
                nc.gpsimd.dma_start(out=wv_sb[:, 2:ND, :], in_=Wvb[:, 2:ND, :])

                for sc in range(S // CW):
                    if sc == 0:
                        xs = xs0
                    elif sc == 1:
                        xs = xs1
                    else:
                        xs = xp1.tile([P, ND, CW], FP8, tag="xs", name=f"xs{sc}")
                        nc.sync.dma_start(out=xs, in_=xTb[:, :, ts(sc, CW)])
                    # zT[:, i, sc] = sum_g G-pair[g]^T @ x-pair[g]   (DR)
                    # [P,256] psums are packed two-per-bank ([P,2,CW] tiles,
                    # bufs=2): 4 rotation entries in 2 PSUM banks
                    for i in range(ND):
                        if i % 2 == 0:
                            ps2 = p256.tile([P, 2, CW], F32, tag="mm",
                                            name=f"zps{i}_{sc}")
                        ps = ps2[:, i % 2, :]
                        for g in range(ND // 2):
                            nc.tensor.matmul(ps, gp[:, 2 * g:2 * g + 2, ts(i, P)],
                                             xs[:, 2 * g:2 * g + 2, :],
                                             start=(g == 0), stop=(g == ND // 2 - 1),
                                             perf_mode=DR)
                        # ScalarE is otherwise idle in phase 1; draining z
                        # there keeps DVE free for the v drains
                        nc.scalar.activation(out=zT[:, i, ts(sc, CW)], in_=ps,
                                             func=AF.Copy)
                    # v[:, t, ec] = sum_g x-pair[g][:,tl]^T @ wv-pair[g][:,ec] (DR)
                    for tl in range(M2):
                        t_ = sc * M2 + tl
                        for ec in range(2):
                            ps = p512.tile([P, 512], F32, tag="mm",
                                           name=f"vps{t_}_{ec}")
                            for g in range(ND // 2):
                                nc.tensor.matmul(
                                    ps, xs[:, 2 * g:2 * g + 2, ts(tl, P)],
                                    wv_sb[:, 2 * g:2 * g + 2, ts(ec, 512)],
                                    start=(g == 0), stop=(g == ND // 2 - 1),
                                    perf_mode=DR)
                            nc.vector.tensor_copy(out=v[:, t_, ts(ec, 512)], in_=ps)
                # chunk-0 attention inputs, then the FFN weights, behind
                # the phase-1 stream (all needed only from ~60us on)
                xc0 = xTcp.tile([P, ND, CW], FP8, tag="xc", name="xc0")
                nc.sync.dma_start(out=xc0, in_=xTb[:, :, ts(0, CW)])
                xm0 = xmp.tile([P, M2, D], BF16, tag="xm", name="xm0")
                nc.sync.dma_start(
                    out=xm0,
                    in_=x_res.rearrange("(t p) d -> p t d", p=P)[:, 0:M2, :])
                nc.sync.dma_start(out=w1p,
                                  in_=W1T.rearrange("(j p) h -> p j h", p=P))
                nc.sync.dma_start(out=w2p,
                                  in_=W2T.rearrange("(n p) d -> p n d", p=P))

            # ======== chunk loop. PE emission order per iteration:
            # scores(c) -> FFN1(c-1) -> PV(c) -> FFN2(c-1) (m-sequential,
            # hT transposes of chunk c interleaved between the m-groups).
            # The exp drains of scores(c) complete under FFN1(c-1)'s MMs and
            # the relu drains of FFN1(c-1) under PV(c)'s, so neither gates
            # the PE stream. ====
            with (
                tc.tile_pool(name="PT", bufs=2) as PTp,
                tc.tile_pool(name="hTc", bufs=2) as hTcp,
                tc.tile_pool(name="hn16", bufs=4) as hn16p,
                tc.tile_pool(name="uT", bufs=2) as uTp,
                tc.tile_pool(name="f32s", bufs=12) as f32sp,
            ):
                def rstd_of(mv, nm):
                    # rstd = exp(-0.5*ln(var+eps)); Ln/Exp live in the same
                    # act-table set as Relu/Copy (Sqrt does not and would
                    # force 2 table reloads per chunk)
                    lnv = scal.tile([P, 1], F32, tag="lnv", name=f"lnv{nm}")
                    nc.scalar.activation(out=lnv, in_=mv[:, 1:2],
                                         func=AF.Ln, bias=eps_t)
                    rstd = scal.tile([P, 1], F32, tag="rstd", name=f"rstd{nm}")
                    nc.scalar.activation(out=rstd, in_=lnv,
                                         func=AF.Exp, scale=-0.5)
                    return rstd

                hn_prev = hc_prev = hr_prev = None
                xc_next, xm_next = xc0, xm0
                for c in range(NCH + 1):
                    hn_cur, hr_cur = [], []
                    if c < NCH:
                        # bf16 transpose bank: [:, k%4, :] gives the hT
                        # transposes a 4-deep rotation so transpose(k) only
                        # waits on the drain of transpose(k-4); pdt holds the
                        # PV denominator accumulators (f32 matmul psums)
                        trb = ptr.tile([P, 4, P], BF16, tag="tr", name=f"trb{c}")
                        pdt = ptr.tile([P, 2], F32, tag="pd", name=f"pdt{c}")
                        xc, xm = xc_next, xm_next
                        # ---- prefetch chunk c+1 inputs one iteration ahead --
                        if c + 1 < NCH:
                            xc_next = xTcp.tile([P, ND, CW], FP8, tag="xc",
                                                name=f"xc{c + 1}")
                            nc.sync.dma_start(out=xc_next,
                                              in_=xTb[:, :, ts(c + 1, CW)])
                            xm_next = xmp.tile([P, M2, D], BF16, tag="xm",
                                               name=f"xm{c + 1}")
                            nc.sync.dma_start(
                                out=xm_next,
                                in_=x_res.rearrange(
                                    "(t p) d -> p t d",
                                    p=P)[:, (c + 1) * M2:(c + 1) * M2 + M2, :])
                        # ---- scoresT + exp ----
                        pt = PTp.tile([P, NS, CW], FP8, tag="pt", name=f"pt{c}")
                        for t_ in range(NS):
                            if t_ % 2 == 0:
                                ps2 = p256.tile([P, 2, CW], F32, tag="mm",
                                                name=f"sps{c}_{t_}")
                            ps = ps2[:, t_ % 2, :]
                            for g in range(ND // 2):
                                nc.tensor.matmul(ps, zT[:, 2 * g:2 * g + 2, ts(t_, P)],
                                                 xc[:, 2 * g:2 * g + 2, :],
                                                 start=(g == 0),
                                                 stop=(g == ND // 2 - 1),
                                                 perf_mode=DR)
                            nc.scalar.activation(out=pt[:, t_, :], in_=ps, func=AF.Exp,
                                                 scale=1.0 / (32.0 * SG))
                    if c > 0:
                        cp = c - 1
                        # ---- FFN1(cp): uT = (W1'*SW1)-pairs^T @ hT-pairs (DR);
                        # W1' = W1*diag(g1), b1c = SW1*(b1 + W1@be1) fold LN1's
                        # affine; ut holds SW1*u so the drain is a pure
                        # bias+relu (no descale -> same op on ScalarE or DVE) --
                        ut = uTp.tile([P, NH, CW], FP8, tag="ut", name=f"ut{cp}")
                        for n in range(NH):
                            if n % 2 == 0:
                                ps2 = p256.tile([P, 2, CW], F32, tag="mm",
                                                name=f"ups{cp}_{n}")
                            ps = ps2[:, n % 2, :]
                            for g in range(ND // 2):
                                nc.tensor.matmul(ps, w1p[:, 2 * g:2 * g + 2, ts(n, P)],
                                                 hc_prev[:, 2 * g:2 * g + 2, :],
                                                 start=(g == 0),
                                                 stop=(g == ND // 2 - 1),
                                                 perf_mode=DR)
                            if cp == NCH - 1 and n % 2 == 1:
                                # last chunk: nothing left to hide the relu
                                # stream under, so split it across two engines
                                nc.vector.tensor_scalar(
                                    out=ut[:, n, :], in0=ps,
                                    scalar1=b1col[n], scalar2=0.0,
                                    op0=ALU.add, op1=ALU.max)
                            else:
                                nc.scalar.activation(out=ut[:, n, :], in_=ps,
                                                     func=AF.Relu, bias=b1col[n])
                    if c < NCH:
                        # ---- PV + denom; normalize + residual + LN1 ----
                        for m in range(M2):
                            pa = [p512.tile([P, 512], F32, tag="mm",
                                            name=f"pa{c}_{m}_{ec}")
                                  for ec in range(2)]
                            pd = pdt[:, m:m + 1]
                            for g in range(NS // 2):
                                lhs = pt[:, 2 * g:2 * g + 2, ts(m, P)]
                                nc.tensor.matmul(pa[0], lhs, v[:, 2 * g:2 * g + 2, 0:512],
                                                 start=(g == 0), stop=(g == NS // 2 - 1),
                                                 perf_mode=DR)
                                nc.tensor.matmul(pa[1], lhs, v[:, 2 * g:2 * g + 2, 512:1024],
                                                 start=(g == 0), stop=(g == NS // 2 - 1),
                                                 perf_mode=DR)
                                nc.tensor.matmul(pd, lhs, ones8[:, :, 0:1],
                                                 start=(g == 0), stop=(g == NS // 2 - 1),
                                                 perf_mode=DR)
                            r = scal.tile([P, 1], F32, tag="r", name=f"r{c}_{m}")
                            nc.vector.reciprocal(r, pd)
                            hp = f32sp.tile([P, D], F32, tag="f32", name=f"hp{c}_{m}")
                            for ec in range(2):
                                nc.vector.scalar_tensor_tensor(
                                    out=hp[:, ts(ec, 512)], in0=pa[ec], scalar=r,
                                    in1=xm[:, m, ts(ec, 512)],
                                    op0=ALU.mult, op1=ALU.add)
                            # LN1
                            st = stats.tile([P, 2, 6], F32, tag="st", name=f"st{c}_{m}")
                            for hf in range(2):
                                nc.vector.bn_stats(out=st[:, hf, :],
                                                   in_=hp[:, ts(hf, 512)])
                            mv = scal.tile([P, 2], F32, tag="mv", name=f"mv{c}_{m}")
                            nc.vector.bn_aggr(out=mv, in_=st)
                            rstd = rstd_of(mv, f"1_{c}_{m}")
                            # normalized h in bf16: feeds the (bf16, 1 PE
                            # cycle/row) transposes and the hr residual
                            hn = hn16p.tile([P, D], BF16, tag="hn",
                                            name=f"hn{c}_{m}")
                            for hf in range(2):
                                nc.vector.tensor_scalar(
                                    out=hn[:, ts(hf, 512)], in0=hp[:, ts(hf, 512)],
                                    scalar1=mv[:, 0:1], scalar2=rstd,
                                    op0=ALU.subtract, op1=ALU.mult)
                            hn_cur.append(hn)
                            # natural-layout affine residual: g1*hn + (be1+b2)
                            hrm = f32sp.tile([P, D], F32, tag="f32",
                                             name=f"hr{c}_{m}")
                            nc.gpsimd.tensor_mul(hrm, hn, g1bc)
                            nc.gpsimd.tensor_add(hrm, hrm, c1bc)
                            hr_cur.append(hrm)

                    def transpose_group(jj):
                        # four transposes fill the trb bank in (j,m)-order
                        # matching hc[:, 2j:2j+2, :] contiguously, so ONE
                        # [128,512] cast drains them all and FFN1(c+1)'s DR
                        # pair g is gated by cast g alone
                        for k in range(4):
                            j, m = 2 * jj + k // 2, k % 2
                            nc.tensor.transpose(trb[:, k, :],
                                                hn_cur[m][:, ts(j, P)], ident16)
                        nc.vector.tensor_copy(out=hc[:, 2 * jj:2 * jj + 2, :],
                                              in_=trb)

                    if c < NCH:
                        hc = hTcp.tile([P, ND, CW], FP8, tag="hc", name=f"hc{c}")
                    if c > 0:
                        # ---- FFN2(cp) m-sequential: 2 psum banks per m,
                        # (g, dc) order reuses the ut stationary across both
                        # dc halves; LN2 + affine + store drain eagerly and
                        # hide under the next m / next chunk's MMs ----
                        for m in range(M2):
                            psm = [p512.tile([P, 512], F32, tag="mm",
                                             name=f"fps{cp}_{dc}_{m}")
                                   for dc in range(2)]
                            for g in range(NH // 2):
                                for dc in range(2):
                                    nc.tensor.matmul(
                                        psm[dc],
                                        ut[:, 2 * g:2 * g + 2, ts(m, P)],
                                        w2p[:, 2 * g:2 * g + 2, ts(dc, 512)],
                                        start=(g == 0), stop=(g == NH // 2 - 1),
                                        perf_mode=DR)
                            # interleave chunk-c transposes between the FFN2
                            # m-groups so the psm drain hides under PE work
                            if c < NCH:
                                for jj in range(m * (ND // 4),
                                                (m + 1) * (ND // 4)):
                                    transpose_group(jj)
                            u2m = f32sp.tile([P, D], F32, tag="f32",
                                             name=f"u2{cp}_{m}")
                            for dc in range(2):
                                nc.vector.scalar_tensor_tensor(
                                    out=u2m[:, ts(dc, 512)], in0=psm[dc],
                                    scalar=1.0 / (SW * SW1),
                                    in1=hr_prev[m][:, ts(dc, 512)],
                                    op0=ALU.mult, op1=ALU.add)
                            st = stats.tile([P, 2, 6], F32, tag="st",
                                            name=f"st2{cp}_{m}")
                            for hf in range(2):
                                nc.vector.bn_stats(out=st[:, hf, :],
                                                   in_=u2m[:, ts(hf, 512)])
                            mv = scal.tile([P, 2], F32, tag="mv",
                                           name=f"mv2{cp}_{m}")
                            nc.vector.bn_aggr(out=mv, in_=st)
                            rstd = rstd_of(mv, f"2_{cp}_{m}")
                            nc.vector.tensor_scalar(out=u2m, in0=u2m,
                                                    scalar1=mv[:, 0:1],
                                                    scalar2=rstd,
                                                    op0=ALU.subtract,
                                                    op1=ALU.mult)
                            ot = f32sp.tile([P, D], F32, tag="f32",
                                            name=f"ot{cp}_{m}")
                            eng = nc.vector if cp == NCH - 1 else nc.gpsimd
                            eng.tensor_mul(ot, u2m, g2bc)
                            eng.tensor_add(ot, ot, be2bc)
                            nc.sync.dma_start(out=out[ts(cp * M2 + m, P), :],
                                              in_=ot)
                    elif c < NCH:
                        # c == 0: no FFN work to interleave with
                        for jj in range(ND // 2):
                            transpose_group(jj)
                    if c < NCH:
                        hn_prev, hc_prev, hr_prev = hn_cur, hc, hr_cur

    nc.compile()
    return nc


_CACHE = {}


def _get_nc(S):
    if S not in _CACHE:
        _CACHE[S] = build_nc(S)
    return _CACHE[S]


def _fp8(a):
    return np.clip(np.asarray(a, np.float32), -240.0, 240.0).astype(FP8_NP)


def kernel(x, Wq, Wk, Wv, W1, b1, W2, b2, g1, be1, g2, be2):
    x = np.asarray(x, np.float32)
    B, S, D_ = x.shape
    nc = _get_nc(S)

    def fp8T(a, s):  # transpose + scale + cast to fp8, contiguous
        return _fp8(np.ascontiguousarray(np.asarray(a, np.float32).T) * s)

    Gm = _fp8(np.asarray(Wk, np.float32).T @ np.asarray(Wq, np.float32) * SG)
    # fold LN1's affine into FFN1: u = relu((g1*hn+be1)@W1^T + b1)
    #                                = relu(hn@(W1*diag(g1))^T + (b1 + W1@be1))
    W1f = np.asarray(W1, np.float32) * np.asarray(g1, np.float32)[None, :]
    shared = {
        "G": Gm, "WvT": fp8T(Wv, SV), "W1T": fp8T(W1f, SW1), "W2T": fp8T(W2, SW),
        "b1c": SW1 * (np.asarray(b1, np.float32)
                      + np.asarray(W1, np.float32) @ np.asarray(be1, np.float32)),
        "g1": np.asarray(g1, np.float32),
        "be1": np.asarray(be1, np.float32),
        "c1": np.asarray(be1, np.float32) + np.asarray(b2, np.float32),
        "g2": np.asarray(g2, np.float32),
        "be2": np.asarray(be2, np.float32),
    }
    in_maps = []
    for b in range(B):
        m = dict(shared)
        m["x_res"] = np.ascontiguousarray(x[b]).astype(ml_dtypes.bfloat16)
        m["xT"] = _fp8(np.ascontiguousarray(x[b].T))
        in_maps.append(m)

    res = run_bass_kernel_spmd(nc, in_maps, core_ids=list(range(B)))
    return np.stack([np.asarray(res.results[b]["out"], np.float32)
                     for b in range(B)], axis=0)


# revision 16
# speedup vs baseline: 1.1632x; 1.1632x over previous
"""MiniTransformer layer on 8 TRN2 NeuronCores — fp8 DoubleRow edition.

Data-parallel over batch (B=8 -> one batch element per core, no collectives).
All six big matmul groups (z=x@G, v=x@WvT, scores, PV, FFN1, FFN2) run in
fp8-e4m3 with perf_mode=DoubleRow (K=256 per instruction, ~2x bf16 rate).
PSUM accumulation is f32; residuals/LN are f32, so fp8 error only enters
through matmul operands. Host-side scales keep operands in e4m3 range
(TRN e4m3 max is +-240): G*128, Wv*64, W1*128, W2*128; descaling is folded
into activation scales / stt scalars.

v2 scheduling (vs the 361us baseline):
- chunk-loop PE emission order scores(c) -> FFN1(c-1) -> PV(c) ->
  FFN2(c-1)/transposes(c): PV no longer gated on the exp drain stream and
  FFN2 not on the relu stream (head-of-line blocking in the engine FIFOs
  caused ~1.2us stalls at every chunk boundary).
- rstd = exp(-0.5*ln(var+eps)): keeps ScalarE on ONE act-table set
  ({exp, ln, relu, copy}); AF.Sqrt lives in a different set and forced
  2 ACT_TABLE_LOADs (1.3us each) per chunk.
- LN1 output (hn) is written as bf16 and the hT transposes run in bf16
  (1 PE cycle/row instead of 2 for f32).
- FFN2 is m-sequential with LN2 drained eagerly (no deferral queue); the
  last chunk splits relu drains between ScalarE and DVE and runs the LN2
  affine on DVE so the tail is not GpSimd-serialized.
- phase-1 DMAs are spread across the sync and gpsimd queues (descriptor
  issue is ~0.6-1us per dma_start on one queue and was serializing the
  startup); x_res streams in bf16 (half the bytes).
"""

import sys

try:
    import concourse.bass as bass
except ImportError:  # pragma: no cover - fallback when sitecustomize absent
    sys.path.insert(0, "/opt/trn_rl_repo")
    import concourse.bass as bass

import numpy as np
import ml_dtypes

import concourse.mybir as mybir
import concourse.tile as tile
from concourse import bacc
from concourse.bass import ts
from concourse.bass_utils import run_bass_kernel_spmd
from concourse.masks import make_identity

AF = mybir.ActivationFunctionType
ALU = mybir.AluOpType
F32 = mybir.dt.float32
BF16 = mybir.dt.bfloat16
FP8 = mybir.dt.float8e4
DR = mybir.MatmulPerfMode.DoubleRow
FP8_NP = ml_dtypes.float8_e4m3

P = 128
D = 1024
H = 2048
E = D
ND = D // P            # 8 d-tiles
NH = H // P            # 16 h-tiles
CW = 256               # s-chunk width
M2 = CW // P           # 2 m-subtiles per chunk
EPS = 1e-5
# fp8 is scale-invariant in relative error; scales are chosen only to keep
# every on-chip cast far below the TRN e4m3 Inf threshold (|x| >= 248 -> Inf,
# there is no saturating cast in the datapath).
SG = 64.0              # scale on G (=Wk^T Wq); z*SG ~ N(0,21), 11+ sigma margin
SV = 32.0              # scale on Wv; v*SV ~ N(0,18.5)
SW1 = 32.0             # scale on W1; ut holds SW1*u <= ~115 in fp8
SW = 128.0             # scale on W2 (entries bounded by 2.9 exactly)


def build_nc(S=2048):
    NS = S // P            # s-tiles
    NCH = S // CW          # chunks

    nc = bacc.Bacc("TRN2", target_bir_lowering=False, debug=False, num_devices=8)

    x_res = nc.dram_tensor("x_res", [S, D], BF16, kind="ExternalInput").ap()
    xT = nc.dram_tensor("xT", [D, S], FP8, kind="ExternalInput").ap()
    G = nc.dram_tensor("G", [D, D], FP8, kind="ExternalInput").ap()      # *SG
    WvT = nc.dram_tensor("WvT", [D, E], FP8, kind="ExternalInput").ap()  # *SV
    W1T = nc.dram_tensor("W1T", [D, H], FP8, kind="ExternalInput").ap()  # *SW1
    W2T = nc.dram_tensor("W2T", [H, D], FP8, kind="ExternalInput").ap()  # *SW
    b1c = nc.dram_tensor("b1c", [H], F32, kind="ExternalInput").ap()  # SW1*(b1+W1@be1)
    g1 = nc.dram_tensor("g1", [D], F32, kind="ExternalInput").ap()
    be1 = nc.dram_tensor("be1", [D], F32, kind="ExternalInput").ap()
    c1 = nc.dram_tensor("c1", [D], F32, kind="ExternalInput").ap()  # be1 + b2
    g2 = nc.dram_tensor("g2", [D], F32, kind="ExternalInput").ap()
    be2 = nc.dram_tensor("be2", [D], F32, kind="ExternalInput").ap()
    out = nc.dram_tensor("out", [S, D], F32, kind="ExternalOutput").ap()

    def bcast(ap_1d, n):
        return bass.AP(tensor=ap_1d.tensor, offset=ap_1d.offset, ap=[[0, P], [1, n]])

    def col(ap_1d, j):
        return ap_1d.rearrange("(a b) -> a b", b=1)[ts(j, P), :]

    with tile.TileContext(nc) as tc:
        with (
            tc.tile_pool(name="p256", bufs=2, space="PSUM") as p256,
            tc.tile_pool(name="p512", bufs=4, space="PSUM") as p512,
            tc.tile_pool(name="ptr", bufs=1, space="PSUM") as ptr,
            tc.tile_pool(name="persist", bufs=1) as persist,
            tc.tile_pool(name="scal", bufs=24) as scal,
            tc.tile_pool(name="stats", bufs=8) as stats,
            tc.tile_pool(name="xTc", bufs=2) as xTcp,
            tc.tile_pool(name="xmp", bufs=3) as xmp,
        ):
            # ---- persistent packed tensors (dim1 = k-tile index for DR pairs) --
            zT = persist.tile([P, ND, S], FP8, tag="zT", name="zT")
            v = persist.tile([P, NS, E], FP8, tag="v", name="v")
            w1p = persist.tile([P, ND, H], FP8, tag="w1p", name="w1p")
            w2p = persist.tile([P, NH, D], FP8, tag="w2p", name="w2p")

            # ===== phase 1: zT and v, streaming xT chunk-by-chunk =====
            with (
                tc.tile_pool(name="ph1", bufs=1) as ph1,
                tc.tile_pool(name="xp1", bufs=3) as xp1,
            ):
                # weights go down the gpsimd queue, the xT stream down the
                # sync queue: descriptor issue is ~0.6-1us per dma_start on
                # an engine queue, so splitting the issue across two queues
                # roughly halves the serialization at the head of phase 1.
                # The broadcast constants (needed only from the chunk loop
                # on) are issued AFTER the phase-1 weights.
                xTb = xT.rearrange("(j p) s -> p j s", p=P)
                Gb = G.rearrange("(j p) d -> p j d", p=P)
                Wvb = WvT.rearrange("(j p) e -> p j e", p=P)
                # first k-pair split out of the big transfers so the first
                # z/v matmuls are gated on ~300KB, not a full 1MB tensor
                gp = ph1.tile([P, ND, D], FP8, tag="gp", name="gp")
                nc.gpsimd.dma_start(out=gp[:, 0:2, :], in_=Gb[:, 0:2, :])
                xs0 = xp1.tile([P, ND, CW], FP8, tag="xs", name="xs0")
                nc.sync.dma_start(out=xs0, in_=xTb[:, :, ts(0, CW)])
                xs1 = xp1.tile([P, ND, CW], FP8, tag="xs", name="xs1")
                nc.sync.dma_start(out=xs1, in_=xTb[:, :, ts(1, CW)])
                nc.gpsimd.dma_start(out=gp[:, 2:ND, :], in_=Gb[:, 2:ND, :])
                wv_sb = ph1.tile([P, ND, E], FP8, tag="wv", name="wv")
                nc.gpsimd.dma_start(out=wv_sb[:, 0:2, :], in_=Wvb[:, 0:2, :])
                nc.gpsimd.dma_start(out=wv_sb[:, 2:ND, :], in_=Wvb[:, 2:ND, :])

                # ---- constants (gpsimd queue, behind the phase-1 weights) --
                ident16 = persist.tile([P, P], BF16, tag="ident", name="ident16")
                make_identity(nc, ident16)
                # ones*SV as the PV-denominator rhs: den_psum = SV*sum_k PT.
                # [P, 2, 16] so the DR pair-dim AP step is 16B (ISA:
                # step%16==0); only column 0 is used.
                ones8 = persist.tile([P, 2, 16], FP8, tag="ones", name="ones8")
                nc.vector.memset(ones8, SV)
                eps_t = persist.tile([P, 1], F32, tag="eps", name="eps_t")
                nc.vector.memset(eps_t, EPS)
                scr = persist.tile([P, 1], F32, tag="scr", name="scr")
                g1bc = persist.tile([P, D], BF16, tag="g1bc", name="g1bc")
                nc.gpsimd.dma_start(out=g1bc, in_=bcast(g1, D))
                c1bc = persist.tile([P, D], F32, tag="c1bc", name="c1bc")
                nc.gpsimd.dma_start(out=c1bc, in_=bcast(c1, D))
                g2bc = persist.tile([P, D], BF16, tag="g2bc", name="g2bc")
                nc.gpsimd.dma_start(out=g2bc, in_=bcast(g2, D))
                be2bc = persist.tile([P, D], BF16, tag="be2bc", name="be2bc")
                nc.gpsimd.dma_start(out=be2bc, in_=bcast(be2, D))
                b1col = []
                for n in range(NH):
                    t = persist.tile([P, 1], F32, tag=f"b1c{n}", name=f"b1col{n}")
                    nc.gpsimd.dma_start(out=t, in_=col(b1c, n))
                    b1col.append(t)

                for sc in range(S // CW):
                    if sc == 0:
                        xs = xs0
                    elif sc == 1:
                        xs = xs1
                    else:
                        xs = xp1.tile([P, ND, CW], FP8, tag="xs", name=f"xs{sc}")
                        nc.sync.dma_start(out=xs, in_=xTb[:, :, ts(sc, CW)])
                    # zT[:, i, sc] = sum_g G-pair[g]^T @ x-pair[g]   (DR)
                    # [P,256] psums are packed two-per-bank ([P,2,CW] tiles,
                    # bufs=2): 4 rotation entries in 2 PSUM banks
                    for i in range(ND):
                        if i % 2 == 0:
                            ps2 = p256.tile([P, 2, CW], F32, tag="mm",
                                            name=f"zps{i}_{sc}")
                        ps = ps2[:, i % 2, :]
                        for g in range(ND // 2):
                            nc.tensor.matmul(ps, gp[:, 2 * g:2 * g + 2, ts(i, P)],
                                             xs[:, 2 * g:2 * g + 2, :],
                                             start=(g == 0), stop=(g == ND // 2 - 1),
                                             perf_mode=DR)
                        # ScalarE is otherwise idle in phase 1; draining z
                        # there keeps DVE free for the v drains
                        nc.scalar.activation(out=zT[:, i, ts(sc, CW)], in_=ps,
                                             func=AF.Copy)
                    # v[:, t, ec] = sum_g x-pair[g][:,tl]^T @ wv-pair[g][:,ec] (DR)
                    for tl in range(M2):
                        t_ = sc * M2 + tl
                        for ec in range(2):
                            ps = p512.tile([P, 512], F32, tag="mm",
                                           name=f"vps{t_}_{ec}")
                            for g in range(ND // 2):
                                nc.tensor.matmul(
                                    ps, xs[:, 2 * g:2 * g + 2, ts(tl, P)],
                                    wv_sb[:, 2 * g:2 * g + 2, ts(ec, 512)],
                                    start=(g == 0), stop=(g == ND // 2 - 1),
                                    perf_mode=DR)
                            nc.vector.tensor_copy(out=v[:, t_, ts(ec, 512)], in_=ps)
                # chunk-0 attention inputs, then the FFN weights, behind
                # the phase-1 stream (all needed only from ~60us on)
                xc0 = xTcp.tile([P, ND, CW], FP8, tag="xc", name="xc0")
                nc.sync.dma_start(out=xc0, in_=xTb[:, :, ts(0, CW)])
                xm0 = xmp.tile([P, M2, D], BF16, tag="xm", name="xm0")
                nc.sync.dma_start(
                    out=xm0,
                    in_=x_res.rearrange("(t p) d -> p t d", p=P)[:, 0:M2, :])
                nc.sync.dma_start(out=w1p,
                                  in_=W1T.rearrange("(j p) h -> p j h", p=P))
                nc.sync.dma_start(out=w2p,
                                  in_=W2T.rearrange("(n p) d -> p n d", p=P))

            # ======== chunk loop. PE emission order per iteration:
            # scores(c) -> FFN1(c-1) -> PV(c) -> FFN2(c-1) (m-sequential,
            # hT transposes of chunk c interleaved between the m-groups).
            # The exp drains of scores(c) complete under FFN1(c-1)'s MMs and
            # the relu drains of FFN1(c-1) under PV(c)'s, so neither gates
            # the PE stream. ====
            with (
                tc.tile_pool(name="PT", bufs=2) as PTp,
                tc.tile_pool(name="hTc", bufs=2) as hTcp,
                tc.tile_pool(name="hn16", bufs=4) as hn16p,
                tc.tile_pool(name="uT", bufs=2) as uTp,
                tc.tile_pool(name="f32s", bufs=12) as f32sp,
            ):
                def rstd_of(mv, nm):
                    rstd = scal.tile([P, 1], F32, tag="rstd", name=f"rstd{nm}")
                    nc.scalar.activation(out=rstd, in_=mv[:, 1:2],
                                         func=AF.Sqrt, bias=eps_t)
                    nc.vector.reciprocal(rstd, rstd)
                    return rstd

                hn_prev = hc_prev = hr_prev = None
                xc_next, xm_next = xc0, xm0
                for c in range(NCH + 1):
                    hn_cur, hr_cur = [], []
                    if c < NCH:
                        # bf16 transpose bank: [:, k%4, :] gives the hT
                        # transposes a 4-deep rotation so transpose(k) only
                        # waits on the drain of transpose(k-4); pdt holds the
                        # PV denominator accumulators (f32 matmul psums)
                        trb = ptr.tile([P, 4, P], BF16, tag="tr", name=f"trb{c}")
                        pdt = ptr.tile([P, 2], F32, tag="pd", name=f"pdt{c}")
                        xc, xm = xc_next, xm_next
                        # ---- prefetch chunk c+1 inputs one iteration ahead --
                        if c + 1 < NCH:
                            xc_next = xTcp.tile([P, ND, CW], FP8, tag="xc",
                                                name=f"xc{c + 1}")
                            nc.sync.dma_start(out=xc_next,
                                              in_=xTb[:, :, ts(c + 1, CW)])
                            xm_next = xmp.tile([P, M2, D], BF16, tag="xm",
                                               name=f"xm{c + 1}")
                            nc.sync.dma_start(
                                out=xm_next,
                                in_=x_res.rearrange(
                                    "(t p) d -> p t d",
                                    p=P)[:, (c + 1) * M2:(c + 1) * M2 + M2, :])
                        # ---- scoresT + exp ----
                        pt = PTp.tile([P, NS, CW], FP8, tag="pt", name=f"pt{c}")
                        for t_ in range(NS):
                            if t_ % 2 == 0:
                                ps2 = p256.tile([P, 2, CW], F32, tag="mm",
                                                name=f"sps{c}_{t_}")
                            ps = ps2[:, t_ % 2, :]
                            for g in range(ND // 2):
                                nc.tensor.matmul(ps, zT[:, 2 * g:2 * g + 2, ts(t_, P)],
                                                 xc[:, 2 * g:2 * g + 2, :],
                                                 start=(g == 0),
                                                 stop=(g == ND // 2 - 1),
                                                 perf_mode=DR)
                            nc.scalar.activation(out=pt[:, t_, :], in_=ps, func=AF.Exp,
                                                 scale=1.0 / (32.0 * SG))
                        # dummy sqrt: pulls the {sqrt,relu,copy} act-table
                        # load to right after the exp batch, where ScalarE
                        # has slack — the LN rstd sqrts and the relus then
                        # find the set resident instead of paying the 1.3us
                        # load on the LN1->transpose critical path
                        nc.scalar.activation(out=scr, in_=eps_t, func=AF.Sqrt)
                    if c > 0:
                        cp = c - 1
                        # ---- FFN1(cp): uT = (W1'*SW1)-pairs^T @ hT-pairs (DR);
                        # W1' = W1*diag(g1), b1c = SW1*(b1 + W1@be1) fold LN1's
                        # affine; ut holds SW1*u so the drain is a pure
                        # bias+relu (no descale -> same op on ScalarE or DVE) --
                        ut = uTp.tile([P, NH, CW], FP8, tag="ut", name=f"ut{cp}")
                        for n in range(NH):
                            if n % 2 == 0:
                                ps2 = p256.tile([P, 2, CW], F32, tag="mm",
                                                name=f"ups{cp}_{n}")
                            ps = ps2[:, n % 2, :]
                            for g in range(ND // 2):
                                nc.tensor.matmul(ps, w1p[:, 2 * g:2 * g + 2, ts(n, P)],
                                                 hc_prev[:, 2 * g:2 * g + 2, :],
                                                 start=(g == 0),
                                                 stop=(g == ND // 2 - 1),
                                                 perf_mode=DR)
                            if cp == NCH - 1 and n % 2 == 1:
                                # last chunk: nothing left to hide the relu
                                # stream under, so split it across two engines
                                nc.vector.tensor_scalar(
                                    out=ut[:, n, :], in0=ps,
                                    scalar1=b1col[n], scalar2=0.0,
                                    op0=ALU.add, op1=ALU.max)
                            else:
                                nc.scalar.activation(out=ut[:, n, :], in_=ps,
                                                     func=AF.Relu, bias=b1col[n])
                    if c < NCH:
                        # ---- PV + denom; normalize + residual + LN1 ----
                        for m in range(M2):
                            pa = [p512.tile([P, 512], F32, tag="mm",
                                            name=f"pa{c}_{m}_{ec}")
                                  for ec in range(2)]
                            pd = pdt[:, m:m + 1]
                            for g in range(NS // 2):
                                lhs = pt[:, 2 * g:2 * g + 2, ts(m, P)]
                                nc.tensor.matmul(pa[0], lhs, v[:, 2 * g:2 * g + 2, 0:512],
                                                 start=(g == 0), stop=(g == NS // 2 - 1),
                                                 perf_mode=DR)
                                nc.tensor.matmul(pa[1], lhs, v[:, 2 * g:2 * g + 2, 512:1024],
                                                 start=(g == 0), stop=(g == NS // 2 - 1),
                                                 perf_mode=DR)
                                nc.tensor.matmul(pd, lhs, ones8[:, :, 0:1],
                                                 start=(g == 0), stop=(g == NS // 2 - 1),
                                                 perf_mode=DR)
                            r = scal.tile([P, 1], F32, tag="r", name=f"r{c}_{m}")
                            nc.vector.reciprocal(r, pd)
                            hp = f32sp.tile([P, D], F32, tag="f32", name=f"hp{c}_{m}")
                            for ec in range(2):
                                nc.vector.scalar_tensor_tensor(
                                    out=hp[:, ts(ec, 512)], in0=pa[ec], scalar=r,
                                    in1=xm[:, m, ts(ec, 512)],
                                    op0=ALU.mult, op1=ALU.add)
                            # LN1
                            st = stats.tile([P, 2, 6], F32, tag="st", name=f"st{c}_{m}")
                            for hf in range(2):
                                nc.vector.bn_stats(out=st[:, hf, :],
                                                   in_=hp[:, ts(hf, 512)])
                            mv = scal.tile([P, 2], F32, tag="mv", name=f"mv{c}_{m}")
                            nc.vector.bn_aggr(out=mv, in_=st)
                            rstd = rstd_of(mv, f"1_{c}_{m}")
                            # normalized h in bf16: feeds the (bf16, 1 PE
                            # cycle/row) transposes and the hr residual
                            hn = hn16p.tile([P, D], BF16, tag="hn",
                                            name=f"hn{c}_{m}")
                            for hf in range(2):
                                nc.vector.tensor_scalar(
                                    out=hn[:, ts(hf, 512)], in0=hp[:, ts(hf, 512)],
                                    scalar1=mv[:, 0:1], scalar2=rstd,
                                    op0=ALU.subtract, op1=ALU.mult)
                            hn_cur.append(hn)
                            # natural-layout affine residual: g1*hn + (be1+b2)
                            hrm = f32sp.tile([P, D], F32, tag="f32",
                                             name=f"hr{c}_{m}")
                            nc.gpsimd.tensor_mul(hrm, hn, g1bc)
                            nc.gpsimd.tensor_add(hrm, hrm, c1bc)
                            hr_cur.append(hrm)

                    def transpose_group(jj):
                        # four transposes fill the trb bank in (j,m)-order
                        # matching hc[:, 2j:2j+2, :] contiguously, so ONE
                        # [128,512] cast drains them all and FFN1(c+1)'s DR
                        # pair g is gated by cast g alone
                        for k in range(4):
                            j, m = 2 * jj + k // 2, k % 2
                            nc.tensor.transpose(trb[:, k, :],
                                                hn_cur[m][:, ts(j, P)], ident16)
                        nc.vector.tensor_copy(out=hc[:, 2 * jj:2 * jj + 2, :],
                                              in_=trb)

                    if c < NCH:
                        hc = hTcp.tile([P, ND, CW], FP8, tag="hc", name=f"hc{c}")
                    if c > 0:
                        # ---- FFN2(cp) m-sequential: 2 psum banks per m,
                        # (g, dc) order reuses the ut stationary across both
                        # dc halves; LN2 + affine + store drain eagerly and
                        # hide under the next m / next chunk's MMs ----
                        last = cp == NCH - 1
                        for m in range(M2):
                            psm = [p512.tile([P, 512], F32, tag="mm",
                                             name=f"fps{cp}_{dc}_{m}")
                                   for dc in range(2)]
                            if last:
                                # dc-sequential so psm[0] completes ~1.7us
                                # before psm[1]: its drain starts under the
                                # dc=1 matmuls (nothing else hides the tail)
                                for dc in range(2):
                                    for g in range(NH // 2):
                                        nc.tensor.matmul(
                                            psm[dc],
                                            ut[:, 2 * g:2 * g + 2, ts(m, P)],
                                            w2p[:, 2 * g:2 * g + 2, ts(dc, 512)],
                                            start=(g == 0),
                                            stop=(g == NH // 2 - 1),
                                            perf_mode=DR)
                            else:
                                for g in range(NH // 2):
                                    for dc in range(2):
                                        nc.tensor.matmul(
                                            psm[dc],
                                            ut[:, 2 * g:2 * g + 2, ts(m, P)],
                                            w2p[:, 2 * g:2 * g + 2, ts(dc, 512)],
                                            start=(g == 0),
                                            stop=(g == NH // 2 - 1),
                                            perf_mode=DR)
                            # interleave chunk-c transposes between the FFN2
                            # m-groups so the psm drain hides under PE work
                            if c < NCH:
                                for jj in range(m * (ND // 4),
                                                (m + 1) * (ND // 4)):
                                    transpose_group(jj)
                            u2m = f32sp.tile([P, D], F32, tag="f32",
                                             name=f"u2{cp}_{m}")
                            # (GpSimd has no PSUM port — all psum drains
                            # must stay on DVE/ScalarE)
                            st = stats.tile([P, 2, 6], F32, tag="st",
                                            name=f"st2{cp}_{m}")
                            for dc in range(2):
                                nc.vector.scalar_tensor_tensor(
                                    out=u2m[:, ts(dc, 512)], in0=psm[dc],
                                    scalar=1.0 / (SW * SW1),
                                    in1=hr_prev[m][:, ts(dc, 512)],
                                    op0=ALU.mult, op1=ALU.add)
                                nc.vector.bn_stats(out=st[:, dc, :],
                                                   in_=u2m[:, ts(dc, 512)])
                            mv = scal.tile([P, 2], F32, tag="mv",
                                           name=f"mv2{cp}_{m}")
                            nc.vector.bn_aggr(out=mv, in_=st)
                            rstd = rstd_of(mv, f"2_{cp}_{m}")
                            ot = f32sp.tile([P, D], F32, tag="f32",
                                            name=f"ot{cp}_{m}")
                            if last:
                                # half-granular norm+affine+store on DVE so
                                # the first half streams out while the second
                                # is still normalizing
                                for hf in range(2):
                                    nc.vector.tensor_scalar(
                                        out=u2m[:, ts(hf, 512)],
                                        in0=u2m[:, ts(hf, 512)],
                                        scalar1=mv[:, 0:1], scalar2=rstd,
                                        op0=ALU.subtract, op1=ALU.mult)
                                    nc.vector.tensor_mul(
                                        ot[:, ts(hf, 512)],
                                        u2m[:, ts(hf, 512)],
                                        g2bc[:, ts(hf, 512)])
                                    nc.vector.tensor_add(
                                        ot[:, ts(hf, 512)],
                                        ot[:, ts(hf, 512)],
                                        be2bc[:, ts(hf, 512)])
                                    nc.sync.dma_start(
                                        out=out[ts(cp * M2 + m, P),
                                                ts(hf, 512)],
                                        in_=ot[:, ts(hf, 512)])
                            else:
                                nc.vector.tensor_scalar(out=u2m, in0=u2m,
                                                        scalar1=mv[:, 0:1],
                                                        scalar2=rstd,
                                                        op0=ALU.subtract,
                                                        op1=ALU.mult)
                                nc.gpsimd.tensor_mul(ot, u2m, g2bc)
                                nc.gpsimd.tensor_add(ot, ot, be2bc)
                                nc.sync.dma_start(
                                    out=out[ts(cp * M2 + m, P), :], in_=ot)
                    elif c < NCH:
                        # c == 0: no FFN work to interleave with
                        for jj in range(ND // 2):
                            transpose_group(jj)
                    if c < NCH:
                        hn_prev, hc_prev, hr_prev = hn_cur, hc, hr_cur

    nc.compile()
    return nc


_CACHE = {}


def _get_nc(S):
    if S not in _CACHE:
        _CACHE[S] = build_nc(S)
    return _CACHE[S]


def _fp8(a):
    return np.clip(np.asarray(a, np.float32), -240.0, 240.0).astype(FP8_NP)


def kernel(x, Wq, Wk, Wv, W1, b1, W2, b2, g1, be1, g2, be2):
    x = np.asarray(x, np.float32)
    B, S, D_ = x.shape
    nc = _get_nc(S)

    def fp8T(a, s):  # transpose + scale + cast to fp8, contiguous
        return _fp8(np.ascontiguousarray(np.asarray(a, np.float32).T) * s)

    Gm = _fp8(np.asarray(Wk, np.float32).T @ np.asarray(Wq, np.float32) * SG)
    # fold LN1's affine into FFN1: u = relu((g1*hn+be1)@W1^T + b1)
    #                                = relu(hn@(W1*diag(g1))^T + (b1 + W1@be1))
    W1f = np.asarray(W1, np.float32) * np.asarray(g1, np.float32)[None, :]
    shared = {
        "G": Gm, "WvT": fp8T(Wv, SV), "W1T": fp8T(W1f, SW1), "W2T": fp8T(W2, SW),
        "b1c": SW1 * (np.asarray(b1, np.float32)
                      + np.asarray(W1, np.float32) @ np.asarray(be1, np.float32)),
        "g1": np.asarray(g1, np.float32),
        "be1": np.asarray(be1, np.float32),
        "c1": np.asarray(be1, np.float32) + np.asarray(b2, np.float32),
        "g2": np.asarray(g2, np.float32),
        "be2": np.asarray(be2, np.float32),
    }
    in_maps = []
    for b in range(B):
        m = dict(shared)
        m["x_res"] = np.ascontiguousarray(x[b]).astype(ml_dtypes.bfloat16)
        m["xT"] = _fp8(np.ascontiguousarray(x[b].T))
        in_maps.append(m)

    res = run_bass_kernel_spmd(nc, in_maps, core_ids=list(range(B)))
    return np.stack([np.asarray(res.results[b]["out"], np.float32)
                     for b in range(B)], axis=0)


# revision 24
# speedup vs baseline: 1.2205x; 1.0493x over previous
"""MiniTransformer layer on 8 TRN2 NeuronCores — fp8 DoubleRow edition.

Data-parallel over batch (B=8 -> one batch element per core, no collectives).
All six big matmul groups (z=x@G, v=x@WvT, scores, PV, FFN1, FFN2) run in
fp8-e4m3 with perf_mode=DoubleRow (K=256 per instruction, ~2x bf16 rate).
PSUM accumulation is f32; residuals/LN are f32, so fp8 error only enters
through matmul operands. Host-side scales keep operands in e4m3 range
(TRN e4m3 max is +-240): G*128, Wv*64, W1*128, W2*128; descaling is folded
into activation scales / stt scalars.

v2 scheduling (vs the 361us baseline):
- chunk-loop PE emission order scores(c) -> FFN1(c-1) -> PV(c) ->
  FFN2(c-1)/transposes(c): PV no longer gated on the exp drain stream and
  FFN2 not on the relu stream (head-of-line blocking in the engine FIFOs
  caused ~1.2us stalls at every chunk boundary).
- rstd = exp(-0.5*ln(var+eps)): keeps ScalarE on ONE act-table set
  ({exp, ln, relu, copy}); AF.Sqrt lives in a different set and forced
  2 ACT_TABLE_LOADs (1.3us each) per chunk.
- LN1 output (hn) is written as bf16 and the hT transposes run in bf16
  (1 PE cycle/row instead of 2 for f32).
- FFN2 is m-sequential with LN2 drained eagerly (no deferral queue); the
  last chunk splits relu drains between ScalarE and DVE and runs the LN2
  affine on DVE so the tail is not GpSimd-serialized.
- phase-1 DMAs are spread across the sync and gpsimd queues (descriptor
  issue is ~0.6-1us per dma_start on one queue and was serializing the
  startup); x_res streams in bf16 (half the bytes).
"""

import sys

try:
    import concourse.bass as bass
except ImportError:  # pragma: no cover - fallback when sitecustomize absent
    sys.path.insert(0, "/opt/trn_rl_repo")
    import concourse.bass as bass

import numpy as np
import ml_dtypes

import concourse.mybir as mybir
import concourse.tile as tile
from concourse import bacc
from concourse.bass import ts
from concourse.bass_utils import run_bass_kernel_spmd
from concourse.masks import make_identity

AF = mybir.ActivationFunctionType
ALU = mybir.AluOpType
F32 = mybir.dt.float32
BF16 = mybir.dt.bfloat16
FP8 = mybir.dt.float8e4
DR = mybir.MatmulPerfMode.DoubleRow
FP8_NP = ml_dtypes.float8_e4m3

P = 128
D = 1024
H = 2048
E = D
ND = D // P            # 8 d-tiles
NH = H // P            # 16 h-tiles
CW = 256               # s-chunk width
M2 = CW // P           # 2 m-subtiles per chunk
EPS = 1e-5
# fp8 is scale-invariant in relative error; scales are chosen only to keep
# every on-chip cast far below the TRN e4m3 Inf threshold (|x| >= 248 -> Inf,
# there is no saturating cast in the datapath).
SG = 64.0              # scale on G (=Wk^T Wq); z*SG ~ N(0,21), 11+ sigma margin
SV = 32.0              # scale on Wv; v*SV ~ N(0,18.5)
SW1 = 32.0             # scale on W1; ut holds SW1*u <= ~115 in fp8
SW = 128.0             # scale on W2 (entries bounded by 2.9 exactly)


def build_nc(S=2048):
    NS = S // P            # s-tiles
    NCH = S // CW          # chunks

    nc = bacc.Bacc("TRN2", target_bir_lowering=False, debug=False, num_devices=8)

    # weights and the residual stream are passed PRE-SHUFFLED to
    # partition-major [P, tiles, cols] so each DMA is one 2D descriptor
    # (128 x contiguous-bytes @ fixed stride) instead of 128*tiles strided
    # descriptors — descriptor issue on the engine queues was serializing
    # the startup (~10us for a single x_res chunk in the (t p) d layout)
    x_res = nc.dram_tensor("x_res", [P, NS, D], BF16, kind="ExternalInput").ap()
    xT = nc.dram_tensor("xT", [D, S], FP8, kind="ExternalInput").ap()
    G = nc.dram_tensor("G", [P, ND, D], FP8, kind="ExternalInput").ap()      # *SG
    WvT = nc.dram_tensor("WvT", [P, ND, E], FP8, kind="ExternalInput").ap()  # *SV
    W1T = nc.dram_tensor("W1T", [P, ND, H], FP8, kind="ExternalInput").ap()  # *SW1
    W2T = nc.dram_tensor("W2T", [P, NH, D], FP8, kind="ExternalInput").ap()  # *SW
    b1c = nc.dram_tensor("b1c", [H], F32, kind="ExternalInput").ap()  # SW1*(b1+W1@be1)
    g1 = nc.dram_tensor("g1", [D], F32, kind="ExternalInput").ap()
    be1 = nc.dram_tensor("be1", [D], F32, kind="ExternalInput").ap()
    c1 = nc.dram_tensor("c1", [D], F32, kind="ExternalInput").ap()  # be1 + b2
    g2 = nc.dram_tensor("g2", [D], F32, kind="ExternalInput").ap()
    be2 = nc.dram_tensor("be2", [D], F32, kind="ExternalInput").ap()
    out = nc.dram_tensor("out", [S, D], F32, kind="ExternalOutput").ap()

    def bcast(ap_1d, n):
        return bass.AP(tensor=ap_1d.tensor, offset=ap_1d.offset, ap=[[0, P], [1, n]])

    def col(ap_1d, j):
        return ap_1d.rearrange("(a b) -> a b", b=1)[ts(j, P), :]

    with tile.TileContext(nc) as tc:
        with (
            tc.tile_pool(name="p256", bufs=3, space="PSUM") as p256,
            tc.tile_pool(name="p512", bufs=3, space="PSUM") as p512,
            tc.tile_pool(name="ptr", bufs=1, space="PSUM") as ptr,
            tc.tile_pool(name="persist", bufs=1) as persist,
            tc.tile_pool(name="scal", bufs=24) as scal,
            tc.tile_pool(name="stats", bufs=8) as stats,
            tc.tile_pool(name="xTc", bufs=2) as xTcp,
            tc.tile_pool(name="xmp", bufs=3) as xmp,
        ):
            # ---- persistent packed tensors (dim1 = k-tile index for DR pairs) --
            zT = persist.tile([P, ND, S], FP8, tag="zT", name="zT")
            v = persist.tile([P, NS, E], FP8, tag="v", name="v")
            w1p = persist.tile([P, ND, H], FP8, tag="w1p", name="w1p")
            w2p = persist.tile([P, NH, D], FP8, tag="w2p", name="w2p")

            # ===== phase 1: zT and v, streaming xT chunk-by-chunk =====
            with (
                tc.tile_pool(name="ph1", bufs=1) as ph1,
                tc.tile_pool(name="xp1", bufs=3) as xp1,
            ):
                # weights go down the gpsimd queue, the xT stream down the
                # sync queue: descriptor issue is ~0.6-1us per dma_start on
                # an engine queue, so splitting the issue across two queues
                # roughly halves the serialization at the head of phase 1.
                # The broadcast constants (needed only from the chunk loop
                # on) are issued AFTER the phase-1 weights.
                xTb = xT.rearrange("(j p) s -> p j s", p=P)
                Gb = G
                Wvb = WvT
                # first k-pair split out of the big transfers so the first
                # z/v matmuls are gated on ~300KB, not a full 1MB tensor
                gp = ph1.tile([P, ND, D], FP8, tag="gp", name="gp")
                nc.gpsimd.dma_start(out=gp[:, 0:2, :], in_=Gb[:, 0:2, :])
                xs0 = xp1.tile([P, ND, CW], FP8, tag="xs", name="xs0")
                nc.sync.dma_start(out=xs0, in_=xTb[:, :, ts(0, CW)])
                xs1 = xp1.tile([P, ND, CW], FP8, tag="xs", name="xs1")
                nc.sync.dma_start(out=xs1, in_=xTb[:, :, ts(1, CW)])
                nc.gpsimd.dma_start(out=gp[:, 2:ND, :], in_=Gb[:, 2:ND, :])
                wv_sb = ph1.tile([P, ND, E], FP8, tag="wv", name="wv")
                nc.gpsimd.dma_start(out=wv_sb[:, 0:2, :], in_=Wvb[:, 0:2, :])
                nc.gpsimd.dma_start(out=wv_sb[:, 2:ND, :], in_=Wvb[:, 2:ND, :])

                # ---- constants (gpsimd queue, behind the phase-1 weights) --
                ident16 = persist.tile([P, P], BF16, tag="ident", name="ident16")
                make_identity(nc, ident16)
                # ones*SV as the PV-denominator rhs: den_psum = SV*sum_k PT.
                # [P, 2, 16] so the DR pair-dim AP step is 16B (ISA:
                # step%16==0); only column 0 is used.
                ones8 = persist.tile([P, 2, 16], FP8, tag="ones", name="ones8")
                nc.vector.memset(ones8, SV)
                eps_t = persist.tile([P, 1], F32, tag="eps", name="eps_t")
                nc.vector.memset(eps_t, EPS)
                scr = persist.tile([P, 1], F32, tag="scr", name="scr")
                g1bc = persist.tile([P, D], BF16, tag="g1bc", name="g1bc")
                nc.gpsimd.dma_start(out=g1bc, in_=bcast(g1, D))
                c1bc = persist.tile([P, D], F32, tag="c1bc", name="c1bc")
                nc.gpsimd.dma_start(out=c1bc, in_=bcast(c1, D))
                g2bc = persist.tile([P, D], BF16, tag="g2bc", name="g2bc")
                nc.gpsimd.dma_start(out=g2bc, in_=bcast(g2, D))
                be2bc = persist.tile([P, D], BF16, tag="be2bc", name="be2bc")
                nc.gpsimd.dma_start(out=be2bc, in_=bcast(be2, D))
                b1col = []
                for n in range(NH):
                    t = persist.tile([P, 1], F32, tag=f"b1c{n}", name=f"b1col{n}")
                    nc.gpsimd.dma_start(out=t, in_=col(b1c, n))
                    b1col.append(t)

                for sc in range(S // CW):
                    if sc == 0:
                        xs = xs0
                    elif sc == 1:
                        xs = xs1
                    else:
                        xs = xp1.tile([P, ND, CW], FP8, tag="xs", name=f"xs{sc}")
                        nc.sync.dma_start(out=xs, in_=xTb[:, :, ts(sc, CW)])
                    # zT[:, i, sc] = sum_g G-pair[g]^T @ x-pair[g]   (DR)
                    for i in range(ND):
                        ps = p256.tile([P, CW], F32, tag="mm", name=f"zps{i}_{sc}")
                        for g in range(ND // 2):
                            nc.tensor.matmul(ps, gp[:, 2 * g:2 * g + 2, ts(i, P)],
                                             xs[:, 2 * g:2 * g + 2, :],
                                             start=(g == 0), stop=(g == ND // 2 - 1),
                                             perf_mode=DR)
                        # ScalarE is otherwise idle in phase 1; draining z
                        # there keeps DVE free for the v drains
                        nc.scalar.activation(out=zT[:, i, ts(sc, CW)], in_=ps,
                                             func=AF.Copy)
                    # v[:, t, ec] = sum_g x-pair[g][:,tl]^T @ wv-pair[g][:,ec] (DR)
                    for tl in range(M2):
                        t_ = sc * M2 + tl
                        for ec in range(2):
                            ps = p512.tile([P, 512], F32, tag="mm",
                                           name=f"vps{t_}_{ec}")
                            for g in range(ND // 2):
                                nc.tensor.matmul(
                                    ps, xs[:, 2 * g:2 * g + 2, ts(tl, P)],
                                    wv_sb[:, 2 * g:2 * g + 2, ts(ec, 512)],
                                    start=(g == 0), stop=(g == ND // 2 - 1),
                                    perf_mode=DR)
                            nc.vector.tensor_copy(out=v[:, t_, ts(ec, 512)], in_=ps)
                # chunk-0 attention inputs, then the FFN weights, behind
                # the phase-1 stream (all needed only from ~60us on)
                xc0 = xTcp.tile([P, ND, CW], FP8, tag="xc", name="xc0")
                nc.sync.dma_start(out=xc0, in_=xTb[:, :, ts(0, CW)])
                xm0 = xmp.tile([P, M2, D], BF16, tag="xm", name="xm0")
                nc.sync.dma_start(out=xm0, in_=x_res[:, 0:M2, :])
                nc.sync.dma_start(out=w1p, in_=W1T)
                nc.sync.dma_start(out=w2p, in_=W2T)

            # ======== chunk loop. PE emission order per iteration:
            # scores(c) -> FFN1(c-1) -> PV(c) -> FFN2(c-1) (m-sequential,
            # hT transposes of chunk c interleaved between the m-groups).
            # The exp drains of scores(c) complete under FFN1(c-1)'s MMs and
            # the relu drains of FFN1(c-1) under PV(c)'s, so neither gates
            # the PE stream. ====
            with (
                tc.tile_pool(name="PT", bufs=2) as PTp,
                tc.tile_pool(name="hTc", bufs=2) as hTcp,
                tc.tile_pool(name="hn16", bufs=4) as hn16p,
                tc.tile_pool(name="uT", bufs=2) as uTp,
                tc.tile_pool(name="f32s", bufs=12) as f32sp,
            ):
                def rstd_of(mv, nm):
                    rstd = scal.tile([P, 1], F32, tag="rstd", name=f"rstd{nm}")
                    nc.scalar.activation(out=rstd, in_=mv[:, 1:2],
                                         func=AF.Sqrt, bias=eps_t)
                    nc.vector.reciprocal(rstd, rstd)
                    return rstd

                hn_prev = hc_prev = hr_prev = None
                xc_next, xm_next = xc0, xm0
                for c in range(NCH + 1):
                    hn_cur, hr_cur = [], []
                    if c < NCH:
                        # bf16 transpose bank: [:, k%4, :] gives the hT
                        # transposes a 4-deep rotation so transpose(k) only
                        # waits on the drain of transpose(k-4); pdt holds the
                        # PV denominator accumulators (f32 matmul psums)
                        trb = ptr.tile([P, 4, P], BF16, tag="tr", name=f"trb{c}")
                        pdt = ptr.tile([P, 2], F32, tag="pd", name=f"pdt{c}")
                        xc, xm = xc_next, xm_next
                        # ---- prefetch chunk c+1 inputs one iteration ahead --
                        if c + 1 < NCH:
                            xc_next = xTcp.tile([P, ND, CW], FP8, tag="xc",
                                                name=f"xc{c + 1}")
                            nc.sync.dma_start(out=xc_next,
                                              in_=xTb[:, :, ts(c + 1, CW)])
                            xm_next = xmp.tile([P, M2, D], BF16, tag="xm",
                                               name=f"xm{c + 1}")
                            nc.sync.dma_start(
                                out=xm_next,
                                in_=x_res[:, (c + 1) * M2:(c + 1) * M2 + M2, :])
                        # ---- scoresT + exp ----
                        pt = PTp.tile([P, NS, CW], FP8, tag="pt", name=f"pt{c}")
                        for t_ in range(NS):
                            ps = p256.tile([P, CW], F32, tag="mm", name=f"sps{c}_{t_}")
                            for g in range(ND // 2):
                                nc.tensor.matmul(ps, zT[:, 2 * g:2 * g + 2, ts(t_, P)],
                                                 xc[:, 2 * g:2 * g + 2, :],
                                                 start=(g == 0),
                                                 stop=(g == ND // 2 - 1),
                                                 perf_mode=DR)
                            nc.scalar.activation(out=pt[:, t_, :], in_=ps, func=AF.Exp,
                                                 scale=1.0 / (32.0 * SG))
                        # dummy sqrt: pulls the {sqrt,relu,copy} act-table
                        # load to right after the exp batch, where ScalarE
                        # has slack — the LN rstd sqrts and the relus then
                        # find the set resident instead of paying the 1.3us
                        # load on the LN1->transpose critical path
                        nc.scalar.activation(out=scr, in_=pt[:, NS - 1, 0:1],
                                             func=AF.Sqrt)
                    if c > 0:
                        cp = c - 1
                        # ---- FFN1(cp): uT = (W1'*SW1)-pairs^T @ hT-pairs (DR);
                        # W1' = W1*diag(g1), b1c = SW1*(b1 + W1@be1) fold LN1's
                        # affine; ut holds SW1*u so the drain is a pure
                        # bias+relu (no descale -> same op on ScalarE or DVE) --
                        ut = uTp.tile([P, NH, CW], FP8, tag="ut", name=f"ut{cp}")
                        for n in range(NH):
                            ps = p256.tile([P, CW], F32, tag="mm", name=f"ups{cp}_{n}")
                            for g in range(ND // 2):
                                nc.tensor.matmul(ps, w1p[:, 2 * g:2 * g + 2, ts(n, P)],
                                                 hc_prev[:, 2 * g:2 * g + 2, :],
                                                 start=(g == 0),
                                                 stop=(g == ND // 2 - 1),
                                                 perf_mode=DR)
                            if cp == NCH - 1 and n % 2 == 1:
                                # last chunk: nothing left to hide the relu
                                # stream under, so split it across two engines
                                nc.vector.tensor_scalar(
                                    out=ut[:, n, :], in0=ps,
                                    scalar1=b1col[n], scalar2=0.0,
                                    op0=ALU.add, op1=ALU.max)
                            else:
                                nc.scalar.activation(out=ut[:, n, :], in_=ps,
                                                     func=AF.Relu, bias=b1col[n])
                    if c < NCH:
                        # ---- PV + denom; normalize + residual + LN1 ----
                        for m in range(M2):
                            pa = [p512.tile([P, 512], F32, tag="mm",
                                            name=f"pa{c}_{m}_{ec}")
                                  for ec in range(2)]
                            pd = pdt[:, m:m + 1]
                            for g in range(NS // 2):
                                lhs = pt[:, 2 * g:2 * g + 2, ts(m, P)]
                                nc.tensor.matmul(pa[0], lhs, v[:, 2 * g:2 * g + 2, 0:512],
                                                 start=(g == 0), stop=(g == NS // 2 - 1),
                                                 perf_mode=DR)
                                nc.tensor.matmul(pa[1], lhs, v[:, 2 * g:2 * g + 2, 512:1024],
                                                 start=(g == 0), stop=(g == NS // 2 - 1),
                                                 perf_mode=DR)
                                nc.tensor.matmul(pd, lhs, ones8[:, :, 0:1],
                                                 start=(g == 0), stop=(g == NS // 2 - 1),
                                                 perf_mode=DR)
                            r = scal.tile([P, 1], F32, tag="r", name=f"r{c}_{m}")
                            nc.vector.reciprocal(r, pd)
                            hp = f32sp.tile([P, D], F32, tag="f32", name=f"hp{c}_{m}")
                            for ec in range(2):
                                nc.vector.scalar_tensor_tensor(
                                    out=hp[:, ts(ec, 512)], in0=pa[ec], scalar=r,
                                    in1=xm[:, m, ts(ec, 512)],
                                    op0=ALU.mult, op1=ALU.add)
                            # LN1
                            st = stats.tile([P, 2, 6], F32, tag="st", name=f"st{c}_{m}")
                            for hf in range(2):
                                nc.vector.bn_stats(out=st[:, hf, :],
                                                   in_=hp[:, ts(hf, 512)])
                            mv = scal.tile([P, 2], F32, tag="mv", name=f"mv{c}_{m}")
                            nc.vector.bn_aggr(out=mv, in_=st)
                            rstd = rstd_of(mv, f"1_{c}_{m}")
                            # normalized h in bf16: feeds the (bf16, 1 PE
                            # cycle/row) transposes and the hr residual
                            hn = hn16p.tile([P, D], BF16, tag="hn",
                                            name=f"hn{c}_{m}")
                            for hf in range(2):
                                nc.vector.tensor_scalar(
                                    out=hn[:, ts(hf, 512)], in0=hp[:, ts(hf, 512)],
                                    scalar1=mv[:, 0:1], scalar2=rstd,
                                    op0=ALU.subtract, op1=ALU.mult)
                            hn_cur.append(hn)
                            # natural-layout affine residual: g1*hn + (be1+b2)
                            hrm = f32sp.tile([P, D], F32, tag="f32",
                                             name=f"hr{c}_{m}")
                            nc.gpsimd.tensor_mul(hrm, hn, g1bc)
                            nc.gpsimd.tensor_add(hrm, hrm, c1bc)
                            hr_cur.append(hrm)

                    def transpose_group(jj):
                        # four transposes fill the trb bank in (j,m)-order
                        # matching hc[:, 2j:2j+2, :] contiguously, so ONE
                        # [128,512] cast drains them all and FFN1(c+1)'s DR
                        # pair g is gated by cast g alone
                        for k in range(4):
                            j, m = 2 * jj + k // 2, k % 2
                            nc.tensor.transpose(trb[:, k, :],
                                                hn_cur[m][:, ts(j, P)], ident16)
                        nc.vector.tensor_copy(out=hc[:, 2 * jj:2 * jj + 2, :],
                                              in_=trb)

                    if c < NCH:
                        hc = hTcp.tile([P, ND, CW], FP8, tag="hc", name=f"hc{c}")
                    if c > 0:
                        # ---- FFN2(cp) m-sequential: 2 psum banks per m,
                        # (g, dc) order reuses the ut stationary across both
                        # dc halves; LN2 + affine + store drain eagerly and
                        # hide under the next m / next chunk's MMs ----
                        last = cp == NCH - 1
                        for m in range(M2):
                            psm = [p512.tile([P, 512], F32, tag="mm",
                                             name=f"fps{cp}_{dc}_{m}")
                                   for dc in range(2)]
                            if last:
                                # dc-sequential so psm[0] completes ~1.7us
                                # before psm[1]: its drain starts under the
                                # dc=1 matmuls (nothing else hides the tail)
                                for dc in range(2):
                                    for g in range(NH // 2):
                                        nc.tensor.matmul(
                                            psm[dc],
                                            ut[:, 2 * g:2 * g + 2, ts(m, P)],
                                            w2p[:, 2 * g:2 * g + 2, ts(dc, 512)],
                                            start=(g == 0),
                                            stop=(g == NH // 2 - 1),
                                            perf_mode=DR)
                            else:
                                for g in range(NH // 2):
                                    for dc in range(2):
                                        nc.tensor.matmul(
                                            psm[dc],
                                            ut[:, 2 * g:2 * g + 2, ts(m, P)],
                                            w2p[:, 2 * g:2 * g + 2, ts(dc, 512)],
                                            start=(g == 0),
                                            stop=(g == NH // 2 - 1),
                                            perf_mode=DR)
                            # interleave chunk-c transposes between the FFN2
                            # m-groups so the psm drain hides under PE work
                            if c < NCH:
                                for jj in range(m * (ND // 4),
                                                (m + 1) * (ND // 4)):
                                    transpose_group(jj)
                            u2m = f32sp.tile([P, D], F32, tag="f32",
                                             name=f"u2{cp}_{m}")
                            # (GpSimd has no PSUM port — all psum drains
                            # must stay on DVE/ScalarE)
                            st = stats.tile([P, 2, 6], F32, tag="st",
                                            name=f"st2{cp}_{m}")
                            for dc in range(2):
                                nc.vector.scalar_tensor_tensor(
                                    out=u2m[:, ts(dc, 512)], in0=psm[dc],
                                    scalar=1.0 / (SW * SW1),
                                    in1=hr_prev[m][:, ts(dc, 512)],
                                    op0=ALU.mult, op1=ALU.add)
                                nc.vector.bn_stats(out=st[:, dc, :],
                                                   in_=u2m[:, ts(dc, 512)])
                            mv = scal.tile([P, 2], F32, tag="mv",
                                           name=f"mv2{cp}_{m}")
                            nc.vector.bn_aggr(out=mv, in_=st)
                            rstd = rstd_of(mv, f"2_{cp}_{m}")
                            ot = f32sp.tile([P, D], F32, tag="f32",
                                            name=f"ot{cp}_{m}")
                            if last:
                                # half-granular norm+affine+store on DVE so
                                # the first half streams out while the second
                                # is still normalizing
                                for hf in range(2):
                                    nc.vector.tensor_scalar(
                                        out=u2m[:, ts(hf, 512)],
                                        in0=u2m[:, ts(hf, 512)],
                                        scalar1=mv[:, 0:1], scalar2=rstd,
                                        op0=ALU.subtract, op1=ALU.mult)
                                    nc.vector.tensor_mul(
                                        ot[:, ts(hf, 512)],
                                        u2m[:, ts(hf, 512)],
                                        g2bc[:, ts(hf, 512)])
                                    nc.vector.tensor_add(
                                        ot[:, ts(hf, 512)],
                                        ot[:, ts(hf, 512)],
                                        be2bc[:, ts(hf, 512)])
                                    nc.sync.dma_start(
                                        out=out[ts(cp * M2 + m, P),
                                                ts(hf, 512)],
                                        in_=ot[:, ts(hf, 512)])
                            else:
                                nc.vector.tensor_scalar(out=u2m, in0=u2m,
                                                        scalar1=mv[:, 0:1],
                                                        scalar2=rstd,
                                                        op0=ALU.subtract,
                                                        op1=ALU.mult)
                                nc.gpsimd.tensor_mul(ot, u2m, g2bc)
                                nc.gpsimd.tensor_add(ot, ot, be2bc)
                                nc.sync.dma_start(
                                    out=out[ts(cp * M2 + m, P), :], in_=ot)
                        if c < NCH - 1:
                            # dummy exp, data-dependent on the last LN2 rstd
                            # (so the scheduler can't hoist it before the
                            # sqrts): pulls the exp-set table load into FFN2
                            # slack instead of ahead of chunk c+1's exps,
                            # which gate the next chunk's scores psum frees
                            nc.scalar.activation(out=scr, in_=rstd,
                                                 func=AF.Exp)
                    elif c < NCH:
                        # c == 0: no FFN work to interleave with
                        for jj in range(ND // 2):
                            transpose_group(jj)
                    if c < NCH:
                        hn_prev, hc_prev, hr_prev = hn_cur, hc, hr_cur

    nc.compile()
    return nc


_CACHE = {}


def _get_nc(S):
    if S not in _CACHE:
        _CACHE[S] = build_nc(S)
    return _CACHE[S]


def _fp8(a):
    return np.clip(np.asarray(a, np.float32), -240.0, 240.0).astype(FP8_NP)


def kernel(x, Wq, Wk, Wv, W1, b1, W2, b2, g1, be1, g2, be2):
    x = np.asarray(x, np.float32)
    B, S, D_ = x.shape
    nc = _get_nc(S)

    def fp8T(a, s):  # transpose + scale + cast to fp8, contiguous
        return _fp8(np.ascontiguousarray(np.asarray(a, np.float32).T) * s)

    def pshuf(a):  # [tiles*128, cols] -> partition-major [128, tiles, cols]
        t = a.shape[0] // P
        return np.ascontiguousarray(a.reshape(t, P, a.shape[1]).transpose(1, 0, 2))

    Gm = _fp8(np.asarray(Wk, np.float32).T @ np.asarray(Wq, np.float32) * SG)
    # fold LN1's affine into FFN1: u = relu((g1*hn+be1)@W1^T + b1)
    #                                = relu(hn@(W1*diag(g1))^T + (b1 + W1@be1))
    W1f = np.asarray(W1, np.float32) * np.asarray(g1, np.float32)[None, :]
    shared = {
        "G": pshuf(Gm), "WvT": pshuf(fp8T(Wv, SV)),
        "W1T": pshuf(fp8T(W1f, SW1)), "W2T": pshuf(fp8T(W2, SW)),
        "b1c": SW1 * (np.asarray(b1, np.float32)
                      + np.asarray(W1, np.float32) @ np.asarray(be1, np.float32)),
        "g1": np.asarray(g1, np.float32),
        "be1": np.asarray(be1, np.float32),
        "c1": np.asarray(be1, np.float32) + np.asarray(b2, np.float32),
        "g2": np.asarray(g2, np.float32),
        "be2": np.asarray(be2, np.float32),
    }
    in_maps = []
    for b in range(B):
        m = dict(shared)
        m["x_res"] = pshuf(x[b]).astype(ml_dtypes.bfloat16)
        m["xT"] = _fp8(np.ascontiguousarray(x[b].T))
        in_maps.append(m)

    res = run_bass_kernel_spmd(nc, in_maps, core_ids=list(range(B)))
    return np.stack([np.asarray(res.results[b]["out"], np.float32)
                     for b in range(B)], axis=0)


# revision 25
# speedup vs baseline: 1.2408x; 1.0166x over previous
"""MiniTransformer layer on 8 TRN2 NeuronCores — fp8 DoubleRow edition.

Data-parallel over batch (B=8 -> one batch element per core, no collectives).
All six big matmul groups (z=x@G, v=x@WvT, scores, PV, FFN1, FFN2) run in
fp8-e4m3 with perf_mode=DoubleRow (K=256 per instruction, ~2x bf16 rate).
PSUM accumulation is f32; residuals/LN are f32, so fp8 error only enters
through matmul operands. Host-side scales keep operands in e4m3 range
(TRN e4m3 max is +-240): G*128, Wv*64, W1*128, W2*128; descaling is folded
into activation scales / stt scalars.

Per-core flow (CW=256 token chunks, FFN software-pipelined one chunk back):
  phase1:  zT[d',s] = (x@G)*sG via DR pairs;  v[s,e]*sV = x @ (Wv*sV)^T
  chunk c: scoresT = zT-pairs^T @ xc-pairs    (DR, PSUM f32 = scores*sG)
           PT = exp(scoresT/(32*sG)) fp8      (ScalarE)
           attn*sV = PT-pairs^T @ v-pairs; den*sV via ones=sV  (DR)
           h = attn/den + x ; LN1             (DVE, f32)
           hT via PE transpose; affine folded into PSUM->SBUF copy -> fp8
           uT = (W1*sW)-pairs^T @ hT-pairs ; relu(psum/sW+b1) fp8  (DR)
           ff*sW = uT-pairs^T @ (W2*sW)-pairs (DR); u2 = psum/sW + hr
           LN2 ; *g2+be2 ; DMA out            (DVE/GpSimd)
"""

import sys

try:
    import concourse.bass as bass
except ImportError:  # pragma: no cover - fallback when sitecustomize absent
    sys.path.insert(0, "/opt/trn_rl_repo")
    import concourse.bass as bass

import numpy as np
import ml_dtypes

import concourse.mybir as mybir
import concourse.tile as tile
from concourse import bacc
from concourse.bass import ts
from concourse.bass_utils import run_bass_kernel_spmd
from concourse.masks import make_identity

AF = mybir.ActivationFunctionType
ALU = mybir.AluOpType
F32 = mybir.dt.float32
BF16 = mybir.dt.bfloat16
FP8 = mybir.dt.float8e4
DR = mybir.MatmulPerfMode.DoubleRow
FP8_NP = ml_dtypes.float8_e4m3

P = 128
D = 1024
H = 2048
E = D
ND = D // P            # 8 d-tiles
NH = H // P            # 16 h-tiles
CW = 256               # s-chunk width
M2 = CW // P           # 2 m-subtiles per chunk
EPS = 1e-5
# fp8 is scale-invariant in relative error; scales are chosen only to keep
# every on-chip cast far below the TRN e4m3 Inf threshold (|x| >= 248 -> Inf,
# there is no saturating cast in the datapath).
SG = 64.0              # scale on G (=Wk^T Wq); z*SG ~ N(0,21), 11+ sigma margin
SV = 32.0              # scale on Wv; v*SV ~ N(0,18.5)
SW1 = 32.0             # scale on W1; ut holds SW1*u <= ~115 in fp8
SW = 128.0             # scale on W2 (entries bounded by 2.9 exactly)


def build_nc(S=2048):
    NS = S // P            # s-tiles
    NCH = S // CW          # chunks

    nc = bacc.Bacc("TRN2", target_bir_lowering=False, debug=False, num_devices=8)

    x_res = nc.dram_tensor("x_res", [S, D], F32, kind="ExternalInput").ap()
    xT = nc.dram_tensor("xT", [D, S], FP8, kind="ExternalInput").ap()
    G = nc.dram_tensor("G", [D, D], FP8, kind="ExternalInput").ap()      # *SG
    WvT = nc.dram_tensor("WvT", [D, E], FP8, kind="ExternalInput").ap()  # *SV
    W1T = nc.dram_tensor("W1T", [D, H], FP8, kind="ExternalInput").ap()  # *SW
    W2T = nc.dram_tensor("W2T", [H, D], FP8, kind="ExternalInput").ap()  # *SW
    b1c = nc.dram_tensor("b1c", [H], F32, kind="ExternalInput").ap()  # b1+W1@be1
    g1 = nc.dram_tensor("g1", [D], F32, kind="ExternalInput").ap()
    be1 = nc.dram_tensor("be1", [D], F32, kind="ExternalInput").ap()
    c1 = nc.dram_tensor("c1", [D], F32, kind="ExternalInput").ap()  # be1 + b2
    g2 = nc.dram_tensor("g2", [D], F32, kind="ExternalInput").ap()
    be2 = nc.dram_tensor("be2", [D], F32, kind="ExternalInput").ap()
    out = nc.dram_tensor("out", [S, D], F32, kind="ExternalOutput").ap()

    def bcast(ap_1d, n):
        return bass.AP(tensor=ap_1d.tensor, offset=ap_1d.offset, ap=[[0, P], [1, n]])

    def col(ap_1d, j):
        return ap_1d.rearrange("(a b) -> a b", b=1)[ts(j, P), :]

    with tile.TileContext(nc) as tc:
        with (
            tc.tile_pool(name="p256", bufs=3, space="PSUM") as p256,
            tc.tile_pool(name="p512", bufs=4, space="PSUM") as p512,
            tc.tile_pool(name="ptr", bufs=1, space="PSUM") as ptr,
            tc.tile_pool(name="persist", bufs=1) as persist,
            tc.tile_pool(name="scal", bufs=24) as scal,
            tc.tile_pool(name="stats", bufs=8) as stats,
            tc.tile_pool(name="xTc", bufs=2) as xTcp,
            tc.tile_pool(name="xmp", bufs=3) as xmp,
        ):
            # ---- constants ----
            ident = persist.tile([P, P], F32, tag="ident", name="ident")
            make_identity(nc, ident)
            # ones*SV as the PV-denominator rhs: den_psum = SV * sum_k PT.
            # [P, 2, 16] so the DR pair-dim AP step is 16B (ISA: step%16==0);
            # only column 0 is used.
            ones8 = persist.tile([P, 2, 16], FP8, tag="ones", name="ones8")
            nc.vector.memset(ones8, SV)
            eps_t = persist.tile([P, 1], F32, tag="eps", name="eps_t")
            nc.vector.memset(eps_t, EPS)
            g1bc = persist.tile([P, D], BF16, tag="g1bc", name="g1bc")
            nc.gpsimd.dma_start(out=g1bc, in_=bcast(g1, D))
            c1bc = persist.tile([P, D], F32, tag="c1bc", name="c1bc")
            nc.gpsimd.dma_start(out=c1bc, in_=bcast(c1, D))
            g2bc = persist.tile([P, D], BF16, tag="g2bc", name="g2bc")
            nc.gpsimd.dma_start(out=g2bc, in_=bcast(g2, D))
            be2bc = persist.tile([P, D], BF16, tag="be2bc", name="be2bc")
            nc.gpsimd.dma_start(out=be2bc, in_=bcast(be2, D))
            b1col = []
            for n in range(NH):
                t = persist.tile([P, 1], F32, tag=f"b1c{n}", name=f"b1col{n}")
                nc.gpsimd.dma_start(out=t, in_=col(b1c, n))
                b1col.append(t)

            # ---- persistent packed tensors (dim1 = k-tile index for DR pairs) --
            zT = persist.tile([P, ND, S], FP8, tag="zT", name="zT")
            v = persist.tile([P, NS, E], FP8, tag="v", name="v")
            w1p = persist.tile([P, ND, H], FP8, tag="w1p", name="w1p")
            w2p = persist.tile([P, NH, D], FP8, tag="w2p", name="w2p")

            # ===== phase 1: zT and v, streaming xT chunk-by-chunk =====
            with (
                tc.tile_pool(name="ph1", bufs=1) as ph1,
                tc.tile_pool(name="xp1", bufs=3) as xp1,
            ):
                # one consolidated 3D-AP DMA per tensor: the sync engine
                # issues descriptors serially at ~600ns each, so many small
                # DMAs throttle phase 1 far below DMA bandwidth
                xTb = xT.rearrange("(j p) s -> p j s", p=P)
                Gb = G.rearrange("(j p) d -> p j d", p=P)
                Wvb = WvT.rearrange("(j p) e -> p j e", p=P)
                # first k-pair split out of the big transfers so the first
                # z/v matmuls are gated on ~300KB, not a full 1MB tensor
                gp = ph1.tile([P, ND, D], FP8, tag="gp", name="gp")
                nc.sync.dma_start(out=gp[:, 0:2, :], in_=Gb[:, 0:2, :])
                xs0 = xp1.tile([P, ND, CW], FP8, tag="xs", name="xs0")
                nc.sync.dma_start(out=xs0, in_=xTb[:, :, ts(0, CW)])
                nc.sync.dma_start(out=gp[:, 2:ND, :], in_=Gb[:, 2:ND, :])
                wv_sb = ph1.tile([P, ND, E], FP8, tag="wv", name="wv")
                nc.sync.dma_start(out=wv_sb[:, 0:2, :], in_=Wvb[:, 0:2, :])
                xs1 = xp1.tile([P, ND, CW], FP8, tag="xs", name="xs1")
                nc.sync.dma_start(out=xs1, in_=xTb[:, :, ts(1, CW)])
                nc.sync.dma_start(out=wv_sb[:, 2:ND, :], in_=Wvb[:, 2:ND, :])

                for sc in range(S // CW):
                    if sc == 0:
                        xs = xs0
                    elif sc == 1:
                        xs = xs1
                    else:
                        xs = xp1.tile([P, ND, CW], FP8, tag="xs", name=f"xs{sc}")
                        nc.sync.dma_start(out=xs, in_=xTb[:, :, ts(sc, CW)])
                    # zT[:, i, sc] = sum_g G-pair[g]^T @ x-pair[g]   (DR)
                    for i in range(ND):
                        ps = p256.tile([P, CW], F32, tag="mm", name=f"zps{i}_{sc}")
                        for g in range(ND // 2):
                            nc.tensor.matmul(ps, gp[:, 2 * g:2 * g + 2, ts(i, P)],
                                             xs[:, 2 * g:2 * g + 2, :],
                                             start=(g == 0), stop=(g == ND // 2 - 1),
                                             perf_mode=DR)
                        # ScalarE is otherwise idle in phase 1; draining z
                        # there keeps DVE free for the v drains
                        nc.scalar.activation(out=zT[:, i, ts(sc, CW)], in_=ps,
                                             func=AF.Copy)
                    # v[:, t, ec] = sum_g x-pair[g][:,tl]^T @ wv-pair[g][:,ec] (DR)
                    for tl in range(M2):
                        t_ = sc * M2 + tl
                        for ec in range(2):
                            ps = p512.tile([P, 512], F32, tag="mm",
                                           name=f"vps{t_}_{ec}")
                            for g in range(ND // 2):
                                nc.tensor.matmul(
                                    ps, xs[:, 2 * g:2 * g + 2, ts(tl, P)],
                                    wv_sb[:, 2 * g:2 * g + 2, ts(ec, 512)],
                                    start=(g == 0), stop=(g == ND // 2 - 1),
                                    perf_mode=DR)
                            nc.vector.tensor_copy(out=v[:, t_, ts(ec, 512)], in_=ps)
                # chunk-0 attention inputs, then the FFN weights, behind
                # the phase-1 stream (all needed only from ~85us on)
                xc0 = xTcp.tile([P, ND, CW], FP8, tag="xc", name="xc0")
                nc.sync.dma_start(out=xc0, in_=xTb[:, :, ts(0, CW)])
                xm0 = xmp.tile([P, M2, D], F32, tag="xm", name="xm0")
                nc.sync.dma_start(
                    out=xm0,
                    in_=x_res.rearrange("(t p) d -> p t d", p=P)[:, 0:M2, :])
                nc.sync.dma_start(out=w1p,
                                  in_=W1T.rearrange("(j p) h -> p j h", p=P))
                nc.sync.dma_start(out=w2p,
                                  in_=W2T.rearrange("(n p) d -> p n d", p=P))

            # ======== chunk loop, software-pipelined: FFN runs one chunk
            # behind attention so the LN1->transpose dependency chain hides
            # under FFN(c-1)'s PE work ====
            with (
                tc.tile_pool(name="PT", bufs=2) as PTp,
                tc.tile_pool(name="hTc", bufs=2) as hTcp,
                tc.tile_pool(name="uT", bufs=2) as uTp,
                tc.tile_pool(name="f32s", bufs=14) as f32sp,
            ):
                hc_prev = hr_prev = None
                ln2q = []  # deferred LN2 tails: (cp, m, u2m, mv)
                for c in range(NCH + 1):
                    hp_cur, hr_cur = [], []
                    if c < NCH:
                        # one PSUM bank, manually sub-sliced: [:, m, 0:1] holds
                        # the PV denominator accumulators (dead before the
                        # transposes run), [:, k%4, :] gives the hT transposes
                        # a 4-deep rotation so transpose(k) only waits on the
                        # drain of transpose(k-4)
                        trb = ptr.tile([P, 4, P], F32, tag="tr", name=f"trb{c}")
                        # ---- prefetches (chunk 0 was loaded during phase 1) --
                        if c == 0:
                            xc, xm = xc0, xm0
                        else:
                            xc = xTcp.tile([P, ND, CW], FP8, tag="xc", name=f"xc{c}")
                            nc.sync.dma_start(out=xc, in_=xTb[:, :, ts(c, CW)])
                            xm = xmp.tile([P, M2, D], F32, tag="xm", name=f"xm{c}")
                            nc.sync.dma_start(
                                out=xm,
                                in_=x_res.rearrange("(t p) d -> p t d",
                                                    p=P)[:, c * M2:c * M2 + M2, :])
                        # ---- scoresT + exp ----
                        pt = PTp.tile([P, NS, CW], FP8, tag="pt", name=f"pt{c}")
                        for t_ in range(NS):
                            ps = p256.tile([P, CW], F32, tag="mm", name=f"sps{c}_{t_}")
                            for g in range(ND // 2):
                                nc.tensor.matmul(ps, zT[:, 2 * g:2 * g + 2, ts(t_, P)],
                                                 xc[:, 2 * g:2 * g + 2, :],
                                                 start=(g == 0),
                                                 stop=(g == ND // 2 - 1),
                                                 perf_mode=DR)
                            nc.scalar.activation(out=pt[:, t_, :], in_=ps, func=AF.Exp,
                                                 scale=1.0 / (32.0 * SG))
                        # ---- PV + denom; normalize + residual + LN1 ----
                        for m in range(M2):
                            pa = [p512.tile([P, 512], F32, tag="mm",
                                            name=f"pa{c}_{m}_{ec}")
                                  for ec in range(2)]
                            pd = trb[:, m, 0:1]
                            for g in range(NS // 2):
                                lhs = pt[:, 2 * g:2 * g + 2, ts(m, P)]
                                nc.tensor.matmul(pa[0], lhs, v[:, 2 * g:2 * g + 2, 0:512],
                                                 start=(g == 0), stop=(g == NS // 2 - 1),
                                                 perf_mode=DR)
                                nc.tensor.matmul(pa[1], lhs, v[:, 2 * g:2 * g + 2, 512:1024],
                                                 start=(g == 0), stop=(g == NS // 2 - 1),
                                                 perf_mode=DR)
                                nc.tensor.matmul(pd, lhs, ones8[:, :, 0:1],
                                                 start=(g == 0), stop=(g == NS // 2 - 1),
                                                 perf_mode=DR)
                            r = scal.tile([P, 1], F32, tag="r", name=f"r{c}_{m}")
                            nc.vector.reciprocal(r, pd)
                            hp = f32sp.tile([P, D], F32, tag="f32", name=f"hp{c}_{m}")
                            for ec in range(2):
                                nc.vector.scalar_tensor_tensor(
                                    out=hp[:, ts(ec, 512)], in0=pa[ec], scalar=r,
                                    in1=xm[:, m, ts(ec, 512)],
                                    op0=ALU.mult, op1=ALU.add)
                            # LN1
                            st = stats.tile([P, 2, 6], F32, tag="st", name=f"st{c}_{m}")
                            for hf in range(2):
                                nc.vector.bn_stats(out=st[:, hf, :],
                                                   in_=hp[:, ts(hf, 512)])
                            mv = scal.tile([P, 2], F32, tag="mv", name=f"mv{c}_{m}")
                            nc.vector.bn_aggr(out=mv, in_=st)
                            rstd = scal.tile([P, 1], F32, tag="rstd",
                                             name=f"rstd{c}_{m}")
                            nc.scalar.activation(out=rstd, in_=mv[:, 1:2],
                                                 func=AF.Sqrt, bias=eps_t)
                            nc.vector.reciprocal(rstd, rstd)
                            # normalize in halves so the first transposes can
                            # start half an op earlier
                            for hf in range(2):
                                nc.vector.tensor_scalar(
                                    out=hp[:, ts(hf, 512)], in0=hp[:, ts(hf, 512)],
                                    scalar1=mv[:, 0:1], scalar2=rstd,
                                    op0=ALU.subtract, op1=ALU.mult)
                            hp_cur.append(hp)
                            # natural-layout affine residual: g1*t1 + (be1+b2)
                            hrm = f32sp.tile([P, D], F32, tag="f32",
                                             name=f"hr{c}_{m}")
                            nc.gpsimd.tensor_mul(hrm, hp, g1bc)
                            nc.gpsimd.tensor_add(hrm, hrm, c1bc)
                            hr_cur.append(hrm)
                    if c > 0:
                        cp = c - 1
                        # ---- FFN1(cp): uT = (W1'*SW1)-pairs^T @ hT-pairs (DR);
                        # W1' = W1*diag(g1), b1' = b1 + W1@be1 fold LN1's
                        # affine so the hT path is a pure cast ----
                        ut = uTp.tile([P, NH, CW], FP8, tag="ut", name=f"ut{cp}")
                        for n in range(NH):
                            ps = p256.tile([P, CW], F32, tag="mm", name=f"ups{cp}_{n}")
                            for g in range(ND // 2):
                                nc.tensor.matmul(ps, w1p[:, 2 * g:2 * g + 2, ts(n, P)],
                                                 hc_prev[:, 2 * g:2 * g + 2, :],
                                                 start=(g == 0),
                                                 stop=(g == ND // 2 - 1),
                                                 perf_mode=DR)
                            nc.scalar.activation(out=ut[:, n, :], in_=ps, func=AF.Relu,
                                                 bias=b1col[n], scale=1.0 / SW1)
                        # ---- FFN2(cp): 4 concurrent psums (dc x m), DR.
                        # Last chunk goes m-sequential so LN2(m0)+store run
                        # under FFN2(m1)'s matmuls instead of after them ----
                        psm = [[p512.tile([P, 512], F32, tag="mm",
                                          name=f"fps{cp}_{dc}_{m}")
                                for m in range(M2)] for dc in range(2)]
                        if cp == NCH - 1:
                            m_groups = [[m] for m in range(M2)]
                        else:
                            m_groups = [list(range(M2))]
                        def drain_ffn2(cp, ms, psm=psm, hrp=hr_prev):
                            for m in ms:
                                u2m = f32sp.tile([P, D], F32, tag="f32",
                                                 name=f"u2{cp}_{m}")
                                for dc in range(2):
                                    nc.vector.scalar_tensor_tensor(
                                        out=u2m[:, ts(dc, 512)], in0=psm[dc][m],
                                        scalar=1.0 / SW,
                                        in1=hrp[m][:, ts(dc, 512)],
                                        op0=ALU.mult, op1=ALU.add)
                                st = stats.tile([P, 2, 6], F32, tag="st",
                                                name=f"st2{cp}_{m}")
                                for hf in range(2):
                                    nc.vector.bn_stats(out=st[:, hf, :],
                                                       in_=u2m[:, ts(hf, 512)])
                                mv = scal.tile([P, 2], F32, tag="mv",
                                               name=f"mv2{cp}_{m}")
                                nc.vector.bn_aggr(out=mv, in_=st)
                                ln2q.append((cp, m, u2m, mv))

                        def flush_ln2(upto):
                            while ln2q and (ln2q[0][0] <= upto):
                                cp2, m, u2m, mv = ln2q.pop(0)
                                rstd = scal.tile([P, 1], F32, tag="rstd",
                                                 name=f"rstd2{cp2}_{m}")
                                nc.scalar.activation(out=rstd, in_=mv[:, 1:2],
                                                     func=AF.Sqrt, bias=eps_t)
                                nc.vector.reciprocal(rstd, rstd)
                                nc.vector.tensor_scalar(out=u2m, in0=u2m,
                                                        scalar1=mv[:, 0:1],
                                                        scalar2=rstd,
                                                        op0=ALU.subtract,
                                                        op1=ALU.mult)
                                ot = f32sp.tile([P, D], F32, tag="f32",
                                                name=f"ot{cp2}_{m}")
                                eng = nc.vector if cp2 == NCH - 1 else nc.gpsimd
                                eng.tensor_mul(ot, u2m, g2bc)
                                eng.tensor_add(ot, ot, be2bc)
                                nc.sync.dma_start(out=out[ts(cp2 * M2 + m, P), :],
                                                  in_=ot)

                        drained = []
                        for mg in m_groups:
                            for g in range(NH // 2):
                                for dc in range(2):
                                    for m in mg:
                                        nc.tensor.matmul(
                                            psm[dc][m],
                                            ut[:, 2 * g:2 * g + 2, ts(m, P)],
                                            w2p[:, 2 * g:2 * g + 2, ts(dc, 512)],
                                            start=(g == 0), stop=(g == NH // 2 - 1),
                                            perf_mode=DR)
                            if cp == NCH - 1:
                                # drain m0 while m1's matmuls are still running
                                drain_ffn2(cp, mg)
                                drained += mg
                        drain_ffn2(cp, [m for m in range(M2) if m not in drained])
                    if c < NCH:
                        # ---- hT transposes, emitted after FFN(c-1) so the
                        # LN1 chain is hidden. Four transposes fill the trb
                        # bank in (j,m)-order matching hc[:, 2j:2j+2, :]
                        # contiguously, so ONE [128,512] cast drains them all
                        # and FFN1(c)'s DR pair g is gated by cast g alone ----
                        hc = hTcp.tile([P, ND, CW], FP8, tag="hc", name=f"hc{c}")
                        for jj in range(ND // 2):
                            for k in range(4):
                                j, m = 2 * jj + k // 2, k % 2
                                nc.tensor.transpose(trb[:, k, :],
                                                    hp_cur[m][:, ts(j, P)], ident)
                            if c == NCH - 1:
                                # last chunk: DVE is busy with LN2(c-1); use
                                # idle ScalarE so the tail FFN isn't gated on
                                # these casts
                                nc.scalar.activation(out=hc[:, 2 * jj:2 * jj + 2, :],
                                                     in_=trb, func=AF.Copy)
                            else:
                                nc.vector.tensor_copy(out=hc[:, 2 * jj:2 * jj + 2, :],
                                                      in_=trb)
                        hc_prev, hr_prev = hc, hr_cur
                    if c > 0:
                        flush_ln2(c - 2)
                flush_ln2(NCH)

    nc.compile()
    return nc


_CACHE = {}


def _get_nc(S):
    if S not in _CACHE:
        _CACHE[S] = build_nc(S)
    return _CACHE[S]


def _fp8(a):
    return np.clip(np.asarray(a, np.float32), -240.0, 240.0).astype(FP8_NP)


def kernel(x, Wq, Wk, Wv, W1, b1, W2, b2, g1, be1, g2, be2):
    x = np.asarray(x, np.float32)
    B, S, D_ = x.shape
    nc = _get_nc(S)

    def fp8T(a, s):  # transpose + scale + cast to fp8, contiguous
        return _fp8(np.ascontiguousarray(np.asarray(a, np.float32).T) * s)

    Gm = _fp8(np.asarray(Wk, np.float32).T @ np.asarray(Wq, np.float32) * SG)
    # fold LN1's affine into FFN1: u = relu((g1*hn+be1)@W1^T + b1)
    #                                = relu(hn@(W1*diag(g1))^T + (b1 + W1@be1))
    W1f = np.asarray(W1, np.float32) * np.asarray(g1, np.float32)[None, :]
    shared = {
        "G": Gm, "WvT": fp8T(Wv, SV), "W1T": fp8T(W1f, SW1), "W2T": fp8T(W2, SW),
        "b1c": np.asarray(b1, np.float32)
        + np.asarray(W1, np.float32) @ np.asarray(be1, np.float32),
        "g1": np.asarray(g1, np.float32),
        "be1": np.asarray(be1, np.float32),
        "c1": np.asarray(be1, np.float32) + np.asarray(b2, np.float32),
        "g2": np.asarray(g2, np.float32),
        "be2": np.asarray(be2, np.float32),
    }
    in_maps = []
    for b in range(B):
        m = dict(shared)
        m["x_res"] = np.ascontiguousarray(x[b])
        m["xT"] = _fp8(np.ascontiguousarray(x[b].T))
        in_maps.append(m)

    res = run_bass_kernel_spmd(nc, in_maps, core_ids=list(range(B)))
    return np.stack([np.asarray(res.results[b]["out"], np.float32)
                     for b in range(B)], axis=0)

